# revision 1
# baseline (speedup 1.0000x reference)
"""Trainium2 Bass kernel for nn_MultiHeadAttention (B=2, S=2048, d_model=1024, H=16).

Sharding (8 cores): data-parallel over B (2) x tensor-parallel over head groups
(4 groups of 4 heads).  Each core computes its head-group's Q/K/V projections
(column-sharded weights), attention for its 4 heads, and a row-parallel
out_proj partial product.  The host sums the 4 partials per batch (the
"all-reduce") and adds the output bias.

All on-chip layouts are transposed ([feature, seq]) so that:
  - scores are computed directly transposed  S_T[k,q] = Kh @ Qh^T  (no P
    transpose needed before P@V),
  - softmax denominators come from ones-vector matmuls (col-tiled 4-way),
  - the PE array is fully packed for dk=64 heads via row/col tile_position
    pairing (auto-derived from AP base partitions),
  - the k-loop is software-pipelined one stage (scores of tile k overlap
    exp and P@V of tile k-1), inputs stream on both HWDGE queues.

Dtypes: inputs/projections and the P,V operands are fp16 (1 cyc/row on
the PE, fp32 PSUM accumulation everywhere); scores and out_proj operands
are float32r (TF32 path, 1 cyc/row at N>=256; note f32r cannot be
col-tiled -- XBUS budget -- which is why the P@V/sums side is fp16).
"""

import sys
import numpy as np

for _p in ("/opt/trn_rl_repo", "/root/.axon_site/_ro/trn_rl_repo"):
    if _p not in sys.path:
        sys.path.append(_p)

D_MODEL = 1024
NUM_HEADS = 16
DK = 64
B = 2
S = 2048
N_CORES = 8
HPC = 4               # heads per core
E = HPC * DK          # 256 features per core
NQ = 512              # q-chunk size
N_QC = S // NQ        # 4 q chunks
N_KT = S // 128       # 16 k tiles
N_DT = D_MODEL // 128  # 8 contraction tiles for projections

_PROGRAM = None
_RUN_KWARGS = {}      # test harness may set {"trace": True}
_LAST_RESULTS = None  # BassKernelResults of the last run


def _build_program():
    import concourse.bass as bass
    import concourse.mybir as mybir
    from concourse import bacc, tile
    from contextlib import ExitStack

    f32 = mybir.dt.float32
    f32r = mybir.dt.float32r
    bf16 = mybir.dt.bfloat16
    fp16 = mybir.dt.float16
    AF = mybir.ActivationFunctionType

    nc = bacc.Bacc("TRN2", target_bir_lowering=False, debug=False,
                   num_devices=N_CORES)

    # Per-core DRAM I/O (transposed activations, pre-sliced weights)
    qT = nc.dram_tensor("qT", [D_MODEL, S], mybir.dt.float16, kind="ExternalInput").ap()
    kT = nc.dram_tensor("kT", [D_MODEL, S], mybir.dt.float16, kind="ExternalInput").ap()
    vT = nc.dram_tensor("vT", [D_MODEL, S], mybir.dt.float16, kind="ExternalInput").ap()
    wq = nc.dram_tensor("wq", [D_MODEL, E], mybir.dt.float16, kind="ExternalInput").ap()
    wk = nc.dram_tensor("wk", [D_MODEL, E], mybir.dt.float16, kind="ExternalInput").ap()
    wv = nc.dram_tensor("wv", [D_MODEL, E], mybir.dt.float16, kind="ExternalInput").ap()
    wo = nc.dram_tensor("wo", [E, D_MODEL], f32r, kind="ExternalInput").ap()
    bq = nc.dram_tensor("bq", [E, 1], f32, kind="ExternalInput").ap()
    bk = nc.dram_tensor("bk", [E, 1], f32, kind="ExternalInput").ap()
    bv = nc.dram_tensor("bv", [E, 1], f32, kind="ExternalInput").ap()
    onesl = nc.dram_tensor("onesl", [1, 64], f32r, kind="ExternalInput").ap()
    onesk = nc.dram_tensor("onesk", [128, 1], mybir.dt.float16,
                           kind="ExternalInput").ap()
    zT = nc.dram_tensor("zT", [D_MODEL, S], f32, kind="ExternalOutput").ap()

    def r(ap):  # operands are natively f32r now
        return ap

    with tile.TileContext(nc) as tc, ExitStack() as ctx:
        persist = ctx.enter_context(tc.tile_pool(name="persist", bufs=1))
        const = ctx.enter_context(tc.tile_pool(name="const", bufs=1))

        # Weights resident in SBUF: [128, n_dt, E]-style views
        wq_sb = persist.tile([128, N_DT, E], fp16, tag="wq", name="wq")
        wk_sb = persist.tile([128, N_DT, E], fp16, tag="wk", name="wk")
        wv_sb = persist.tile([128, N_DT, E], fp16, tag="wv", name="wv")
        wo_sb = persist.tile([128, 2, D_MODEL], f32r, tag="wo", name="wo")
        # wk/wq first (gate the K/Q projections), split across queues;
        # wv/wo stream later behind the K inputs
        nc.sync.dma_start(wk_sb[:], wk.rearrange("(t p) e -> p t e", p=128))
        nc.scalar.dma_start(wq_sb[:], wq.rearrange("(t p) e -> p t e", p=128))
        nc.scalar.dma_start(wv_sb[:], wv.rearrange("(t p) e -> p t e", p=128))
        nc.sync.dma_start(wo_sb[:], wo.rearrange("(t p) e -> p t e", p=128))

        bq_sb = persist.tile([128, 2], f32, tag="bq", name="bq")
        bk_sb = persist.tile([128, 2], f32, tag="bk", name="bk")
        bv_sb = persist.tile([128, 2], f32, tag="bv", name="bv")
        nc.sync.dma_start(bq_sb[:], bq.rearrange("(m p) o -> p (m o)", p=128))
        nc.sync.dma_start(bk_sb[:], bk.rearrange("(m p) o -> p (m o)", p=128))
        nc.sync.dma_start(bv_sb[:], bv.rearrange("(m p) o -> p (m o)", p=128))

        from concourse.masks import make_identity
        ident = const.tile([128, 128], fp16, tag="ident", name="ident")
        make_identity(nc, ident)
        # host-provided constants: ones column (sums lhsT) and the
        # pair-broadcast selector
        ones_k = const.tile([128, 1], fp16, tag="ones_k", name="ones_k")
        ones_l = const.tile([1, 64], f32r, tag="ones_l", name="ones_l")
        nc.sync.dma_start(ones_k[:], onesk)
        nc.sync.dma_start(ones_l[:], onesl)

        # Projection outputs (transposed): pair tensors hold 2 heads each
        qh = [persist.tile([128, S], f32r, tag=f"qh{p}", name=f"qh{p}") for p in range(2)]
        kh = [persist.tile([128, S], f32r, tag=f"kh{p}", name=f"kh{p}") for p in range(2)]
        # Vh non-transposed [k, e], s-tile-major columns
        vh = persist.tile([128, N_KT * E], fp16, tag="vh", name="vh")
        # normalized attention output (transposed), pair tensors
        ot = [persist.tile([128, S], f32r, tag=f"ot{p}", name=f"ot{p}") for p in range(2)]

        stage_a = ExitStack()
        xpool = stage_a.enter_context(tc.tile_pool(name="xpool", bufs=8))
        apsum = stage_a.enter_context(
            tc.tile_pool(name="apsum", bufs=8, space="PSUM"))

        # vhT: transposed V projection [e, s] (bf16), transposed to vh after
        vhT = [persist.tile([128, S], fp16, tag=f"vhT{m}", name=f"vhT{m}")
               for m in range(2)]

        # ---- Stage A: projections (all transposed orientation) ---------
        dma_engines = (nc.sync, nc.scalar)  # two HWDGE queues
        for which, xdram, w_sb, b_sb, dst in (
            ("k", kT, wk_sb, bk_sb, kh),
            ("q", qT, wq_sb, bq_sb, qh),
            ("v", vT, wv_sb, bv_sb, vhT),
        ):
            # ps[m][n]: out rows m*128, cols n*512
            ps = [[apsum.tile([128, 512], f32, tag="aps", name="aps") for n in range(4)]
                  for m in range(2)]
            for d in range(N_DT):
                xt = xpool.tile([128, S], fp16, tag="xt", name="xt")
                dma_engines[d % 2].dma_start(xt[:], xdram[d * 128:(d + 1) * 128, :])
                for m in range(2):
                    lhsT = w_sb[:, d, m * 128:(m + 1) * 128]
                    for n in range(4):
                        nc.tensor.matmul(
                            ps[m][n][:], r(lhsT), r(xt[:, n * 512:(n + 1) * 512]),
                            start=(d == 0), stop=(d == N_DT - 1))
            for m in range(2):
                for n in range(4):
                    nc.vector.tensor_scalar_add(
                        dst[m][:, n * 512:(n + 1) * 512], ps[m][n][:],
                        b_sb[:, m:m + 1])

        # vh[s, e] = vhT^T via PE transposes (4 blocks per psum bank)
        for st in range(N_KT):
            tp = apsum.tile([128, 512], fp16, tag="aps", name="tps")                 if st % 2 == 0 else tp
            for m in range(2):
                j = (st % 2) * 2 + m
                nc.tensor.matmul(
                    tp[:, j * 128:(j + 1) * 128],
                    vhT[m][:, st * 128:(st + 1) * 128], ident[:],
                    is_transpose=True, start=True, stop=True,
                    skip_group_check=True)
                nc.vector.tensor_copy(
                    vh[:, st * E + m * 128: st * E + (m + 1) * 128],
                    tp[:, j * 128:(j + 1) * 128])

        stage_a.close()

        # ---- Stage B: attention + out_proj, per q-chunk ----------------
        scp = ctx.enter_context(tc.tile_pool(name="scp", bufs=2, space="PSUM"))
        outp = ctx.enter_context(tc.tile_pool(name="outp", bufs=2, space="PSUM"))
        sump = ctx.enter_context(tc.tile_pool(name="sump", bufs=1, space="PSUM"))
        zp = ctx.enter_context(tc.tile_pool(name="zp", bufs=1, space="PSUM"))

        ptp = ctx.enter_context(tc.tile_pool(name="ptp", bufs=8))
        rp = ctx.enter_context(tc.tile_pool(name="rp", bufs=6))
        bcp = ctx.enter_context(tc.tile_pool(name="bcp", bufs=3))
        zsb = ctx.enter_context(tc.tile_pool(name="zsb", bufs=4))

        for qc in range(N_QC):
            q0, q1 = qc * NQ, (qc + 1) * NQ
            outs = [outp.tile([128, NQ], f32, tag="outp", name="outp") for _ in range(2)]
            sums = sump.tile([128, NQ], f32, tag="sums", name="sums")

            def pv_sums(kt, pts):
                # P@V + denominator for k-tile kt (pts = pair pt tiles)
                for p in range(2):
                    for j in range(2):
                        h = 2 * p + j
                        lo, hi = j * 64, (j + 1) * 64
                        ptj = pts[p][:, j * NQ:(j + 1) * NQ]
                        # P@V (col-tiled pair: head j -> out partitions j*64)
                        nc.tensor.matmul(
                            outs[p][lo:hi, :],
                            r(vh[:, kt * E + h * 64: kt * E + (h + 1) * 64]),
                            r(ptj), start=(kt == 0), stop=(kt == N_KT - 1),
                            skip_group_check=True)
                        # softmax denominator (col-tiled 4-way, M=1)
                        nc.tensor.matmul(
                            sums[32 * h:32 * h + 1, :], r(ones_k[:]), r(ptj),
                            start=(kt == 0), stop=(kt == N_KT - 1),
                            tile_position=(0, 32 * h), skip_group_check=True)

            # k-loop software-pipelined one stage deep: scores(kt) issue on
            # PE while exp(kt-1) runs on ACT and pv/sums(kt-1) follows.
            prev_pts = None
            for kt in range(N_KT):
                k0 = kt * 128
                scs = []
                for p in range(2):
                    # both heads' scores side by side in one 2-bank psum tile
                    sc = scp.tile([128, 2 * NQ], f32, tag="sc", name="sc")
                    for j in range(2):
                        lo, hi = j * 64, (j + 1) * 64
                        nc.tensor.matmul(
                            sc[:, j * NQ:(j + 1) * NQ],
                            r(kh[p][lo:hi, k0:k0 + 128]),
                            r(qh[p][lo:hi, q0:q1]), start=True, stop=True)
                    scs.append(sc)
                if prev_pts is not None:
                    pv_sums(kt - 1, prev_pts)
                pts = []
                for p in range(2):
                    # one wide exp per pair (amortizes ACT fixed cost)
                    pt = ptp.tile([128, 2 * NQ], fp16, tag="pt", name="pt")
                    nc.scalar.activation(pt[:], scs[p][:], AF.Exp, scale=0.125)
                    pts.append(pt)
                prev_pts = pts
            pv_sums(N_KT - 1, prev_pts)
            # normalize: ot = outs * (1/sums) broadcast across partitions
            for p in range(2):
                bc_sb = bcp.tile([128, NQ], f32, tag="bc_sb", name="bc_sb")
                for j in range(2):
                    h = 2 * p + j
                    rv = rp.tile([1, NQ], f32r, tag="rv", name="rv")
                    with nc.allow_low_precision(reason="tf32 softmax recip"):
                        nc.vector.reciprocal(rv[:], sums[32 * h:32 * h + 1, :])
                    # rank-1 broadcast of 1/sum across 64 partitions (PE);
                    # separate base-0 psum tile (f32r can't col-tile)
                    bc = scp.tile([64, NQ], f32, tag="sc", name="bcps")
                    nc.tensor.matmul(bc[:], ones_l[:], rv[:],
                                     start=True, stop=True)
                    nc.vector.tensor_copy(bc_sb[j * 64:(j + 1) * 64, :], bc[:])
                nc.vector.tensor_mul(ot[p][:, q0:q1], outs[p][:], bc_sb[:])
            # out_proj partial: zT[e, q-chunk]
            for e in range(8):
                pool_, tag_ = (zp, "zps") if e % 2 == 0 else (sump, "sums")
                zps = pool_.tile([128, NQ], f32, tag=tag_, name="zps")
                for c in range(2):
                    nc.tensor.matmul(
                        zps[:], r(wo_sb[:, c, e * 128:(e + 1) * 128]),
                        r(ot[c][:, q0:q1]), start=(c == 0), stop=(c == 1))
                zt_sb = zsb.tile([128, NQ], f32, tag="zt_sb", name="zt_sb")
                nc.vector.tensor_copy(zt_sb[:], zps[:])
                dma_engines[e % 2].dma_start(
                    zT[e * 128:(e + 1) * 128, q0:q1], zt_sb[:])

    nc.compile()
    return nc


def _get_program():
    global _PROGRAM
    if _PROGRAM is None:
        _PROGRAM = _build_program()
    return _PROGRAM


ONESL_NP = None
ONESK_NP = None


def _init_consts():
    global ONESL_NP, ONESK_NP
    if ONESL_NP is None:
        import ml_dtypes
        ONESL_NP = np.ones((1, 64), dtype=np.float32)
        ONESK_NP = np.ones((128, 1), np.float16)


def _make_in_maps(q, k, v, Wq, bq, Wk, bk, Wv, bv, Wo):
    _init_consts()
    f32 = np.float32
    xT = {}
    for b in range(B):
        xT[("q", b)] = np.ascontiguousarray(q[b].T, dtype=np.float16)
        xT[("k", b)] = np.ascontiguousarray(k[b].T, dtype=np.float16)
        xT[("v", b)] = np.ascontiguousarray(v[b].T, dtype=np.float16)
    wslices = {}
    for g in range(4):
        sl = slice(g * E, (g + 1) * E)
        wslices[("wq", g)] = np.ascontiguousarray(Wq[sl, :].T, dtype=np.float16)
        wslices[("wk", g)] = np.ascontiguousarray(Wk[sl, :].T, dtype=np.float16)
        wslices[("wv", g)] = np.ascontiguousarray(Wv[sl, :].T, dtype=np.float16)
        wslices[("wo", g)] = np.ascontiguousarray(Wo[:, sl].T, dtype=f32)
        wslices[("bq", g)] = np.ascontiguousarray(bq[sl].reshape(E, 1), dtype=f32)
        wslices[("bk", g)] = np.ascontiguousarray(bk[sl].reshape(E, 1), dtype=f32)
        wslices[("bv", g)] = np.ascontiguousarray(bv[sl].reshape(E, 1),
                                                   dtype=f32)
    in_maps = []
    for c in range(N_CORES):
        b, g = c // 4, c % 4
        in_maps.append({
            "onesl": ONESL_NP, "onesk": ONESK_NP,
            "qT": xT[("q", b)], "kT": xT[("k", b)], "vT": xT[("v", b)],
            "wq": wslices[("wq", g)], "wk": wslices[("wk", g)],
            "wv": wslices[("wv", g)], "wo": wslices[("wo", g)],
            "bq": wslices[("bq", g)], "bk": wslices[("bk", g)],
            "bv": wslices[("bv", g)],
        })
    return in_maps


def _numpy_fallback(q, k, v, mask, Wq, bq, Wk, bk, Wv, bv, Wo, bo):
    # Only used if mask is not all-True (never the case for this problem).
    def proj(x, W, b_):
        y = x @ W.T + b_
        return y.reshape(B, S, NUM_HEADS, DK).transpose(0, 2, 1, 3)
    qh, kh, vh = proj(q, Wq, bq), proj(k, Wk, bk), proj(v, Wv, bv)
    sc = np.einsum("bhqd,bhkd->bhqk", qh, kh) / np.sqrt(DK)
    sc = np.where(mask, sc, np.float32(-1e9))
    sc = sc - sc.max(-1, keepdims=True)
    p = np.exp(sc)
    p /= p.sum(-1, keepdims=True)
    o = np.einsum("bhqk,bhkd->bhqd", p, vh)
    o = o.transpose(0, 2, 1, 3).reshape(B, S, D_MODEL)
    return (o @ Wo.T + bo).astype(np.float32)


def kernel(q, k, v, mask, Wq, bq, Wk, bk, Wv, bv, Wo, bo):
    q = np.asarray(q, dtype=np.float32)
    k = np.asarray(k, dtype=np.float32)
    v = np.asarray(v, dtype=np.float32)
    Wq, Wk, Wv, Wo = (np.asarray(w, dtype=np.float32) for w in (Wq, Wk, Wv, Wo))
    bq, bk, bv, bo = (np.asarray(x, dtype=np.float32) for x in (bq, bk, bv, bo))
    if not np.all(np.asarray(mask)):
        return _numpy_fallback(q, k, v, np.asarray(mask), Wq, bq, Wk, bk,
                               Wv, bv, Wo, bo)

    from concourse.bass_utils import run_bass_kernel_spmd
    nc = _get_program()
    in_maps = _make_in_maps(q, k, v, Wq, bq, Wk, bk, Wv, bv, Wo)
    res = run_bass_kernel_spmd(nc, in_maps, core_ids=list(range(N_CORES)),
                               **_RUN_KWARGS)
    global _LAST_RESULTS
    _LAST_RESULTS = res
    out = np.empty((B, S, D_MODEL), dtype=np.float32)
    for b in range(B):
        acc = res.results[4 * b]["zT"].astype(np.float32).copy()
        for g in range(1, 4):
            acc += res.results[4 * b + g]["zT"]
        out[b] = acc.T + bo
    return out



# revision 30
# speedup vs baseline: 1.5534x; 1.5534x over previous
"""Trainium2 Bass kernel for nn_MultiHeadAttention (B=2, S=2048, d_model=1024, H=16).

Sharding (8 cores): data-parallel over B (2) x tensor-parallel over head groups
(4 groups of 4 heads).  Each core computes its head-group's Q/K/V projections
(column-sharded weights), attention for its 4 heads, and a row-parallel
out_proj partial product.  The host sums the 4 partials per batch (the
"all-reduce") and adds the output bias.

Cost-model-aware layout (PE time = out-free-size x cycle; K, M are free):
  - Q/K projections land transposed [e, s]; V lands direct [s, e] with its
    bias applied by a rank-1 ones matmul and a constant ones column appended
    per head.
  - scores are [k, q] per 2-head pair into a 2-bank PSUM tile; one exp call
    covers 1024 columns.
  - P@V runs output-small: out [q=128, 65] per (head, q-subtile); column 64
    (against the ones column of V) accumulates the softmax denominator free.
  - normalization is per-partition reciprocal+scale on DVE; out tiles are
    transposed for out_proj by the XBAR dma-transpose; out_proj emits [d, q]
    fp16 partials.
  - engines execute in-order, so the outer iteration is head-pair-major
    (pair 0 for all q-chunks, then pair 1) and projection / out_proj /
    V-projection work is drip-fed into the ACT-bound k-loops via chore slots
    tuned to DMA arrival times.
"""

import sys
import numpy as np

for _p in ("/opt/trn_rl_repo", "/root/.axon_site/_ro/trn_rl_repo"):
    if _p not in sys.path:
        sys.path.append(_p)

D_MODEL = 1024
NUM_HEADS = 16
DK = 64
B = 2
S = 2048
N_CORES = 8
HPC = 4               # heads per core
E = HPC * DK          # 256 features per core
NQ = 512              # q-chunk size
N_QC = S // NQ        # 4 q chunks
N_KT = S // 128       # 16 k tiles
N_DT = D_MODEL // 128  # 8 contraction tiles for projections

_DEBUG = False
_PROGRAM = None
_RUN_KWARGS = {}      # test harness may set {"trace": True}
_LAST_RESULTS = None  # BassKernelResults of the last run


def _build_program():
    import concourse.mybir as mybir
    from concourse import bacc, tile
    from contextlib import ExitStack

    f32 = mybir.dt.float32
    fp16 = mybir.dt.float16
    AF = mybir.ActivationFunctionType

    nc = bacc.Bacc("TRN2", target_bir_lowering=False, debug=False,
                   num_devices=N_CORES)

    qT = nc.dram_tensor("qT", [D_MODEL, S], fp16, kind="ExternalInput").ap()
    kT = nc.dram_tensor("kT", [D_MODEL, S], fp16, kind="ExternalInput").ap()
    vT = nc.dram_tensor("vT", [D_MODEL, S], fp16, kind="ExternalInput").ap()
    wq = nc.dram_tensor("wq", [D_MODEL, E], fp16, kind="ExternalInput").ap()
    wk = nc.dram_tensor("wk", [D_MODEL, E], fp16, kind="ExternalInput").ap()
    wv = nc.dram_tensor("wv", [D_MODEL, E], fp16, kind="ExternalInput").ap()
    wo = nc.dram_tensor("wo", [E, D_MODEL], fp16, kind="ExternalInput").ap()
    bq = nc.dram_tensor("bq", [E, 1], f32, kind="ExternalInput").ap()
    bk = nc.dram_tensor("bk", [E, 1], f32, kind="ExternalInput").ap()
    zT = nc.dram_tensor("zT", [D_MODEL, S], fp16, kind="ExternalOutput").ap()
    dbg = None
    if _DEBUG:
        dbg = {
            "kh0": nc.dram_tensor("dkh0", [128, S], fp16,
                                  kind="ExternalOutput").ap(),
            "qh0": nc.dram_tensor("dqh0", [128, S], fp16,
                                  kind="ExternalOutput").ap(),
            "vh": nc.dram_tensor("dvh", [128, N_KT * HPC * (DK + 1)], fp16,
                                 kind="ExternalOutput").ap(),
            "ot0": nc.dram_tensor("dot0", [128, 4 * E], fp16,
                                  kind="ExternalOutput").ap(),
            "otT0": nc.dram_tensor("dotT0", [128, 2 * NQ], fp16,
                                   kind="ExternalOutput").ap(),
            "pt00": nc.dram_tensor("dpt00", [128, 2 * NQ], fp16,
                                   kind="ExternalOutput").ap(),
            "pt05": nc.dram_tensor("dpt05", [128, 2 * NQ], fp16,
                                   kind="ExternalOutput").ap(),
            "pt10": nc.dram_tensor("dpt10", [128, 2 * NQ], fp16,
                                   kind="ExternalOutput").ap(),
            "pt15": nc.dram_tensor("dpt15", [128, 2 * NQ], fp16,
                                   kind="ExternalOutput").ap(),
            "acc0": nc.dram_tensor("dacc0", [128, 4 * (DK + 1)], f32,
                                   kind="ExternalOutput").ap(),
        }
    zTv = zT.rearrange("(a p) q -> p a q", p=128)  # [128, 8, 2048]

    with tile.TileContext(nc) as tc, ExitStack() as ctx:
        persist = ctx.enter_context(tc.tile_pool(name="persist", bufs=1))
        xp = ctx.enter_context(tc.tile_pool(name="xp", bufs=1))
        ptp = ctx.enter_context(tc.tile_pool(name="ptp", bufs=10))
        otp = ctx.enter_context(tc.tile_pool(name="otp", bufs=4))
        scp = ctx.enter_context(tc.tile_pool(name="scp", bufs=2, space="PSUM"))
        accp = ctx.enter_context(tc.tile_pool(name="accp", bufs=2,
                                              space="PSUM"))
        aux = ctx.enter_context(tc.tile_pool(name="aux", bufs=2, space="PSUM"))

        # ---- persistent SBUF tensors ----------------------------------
        wq_sb = persist.tile([128, N_DT, E], fp16, tag="wq", name="wq")
        wk_sb = persist.tile([128, N_DT, E], fp16, tag="wk", name="wk")
        wv_sb = persist.tile([128, N_DT, E], fp16, tag="wv", name="wv")
        wo_sb = persist.tile([128, 2, D_MODEL], fp16, tag="wo", name="wo")
        bq_sb = persist.tile([128, 2], f32, tag="bq", name="bq")
        bk_sb = persist.tile([128, 2], f32, tag="bk", name="bk")
        ones1 = persist.tile([1, 128], fp16, tag="ones1", name="ones1")
        kh = [persist.tile([128, S], fp16, tag=f"kh{p}", name=f"kh{p}")
              for p in range(2)]
        qh = [persist.tile([128, S], fp16, tag=f"qh{p}", name=f"qh{p}")
              for p in range(2)]
        # V projection [s-tile, head, dk+1]; col 64 is the constant 1.0
        vh = persist.tile([128, N_KT, HPC, DK + 1], fp16, tag="vh", name="vh")

        nc.vector.memset(ones1[:], 1.0)
        nc.vector.memset(vh[:, :, :, DK], 1.0)
        from concourse.masks import make_identity
        ident = persist.tile([128, 128], fp16, tag="ident", name="ident")
        make_identity(nc, ident)

        # ---- input streaming (sync/SP queue, in consumption order) ----
        # one DMA per 512-column group: the SP sequencer costs ~650ns per
        # DMA, so grouped [128, d, cols] transfers beat per-d-tile chunks
        kTr = kT.rearrange("(t p) q -> p t q", p=128)
        qTr = qT.rearrange("(t p) q -> p t q", p=128)
        vTr = vT.rearrange("(t p) q -> p t q", p=128)
        xk0 = xp.tile([128, N_DT, NQ], fp16, tag="xk0", name="xk0")
        xk1 = xp.tile([128, N_DT, NQ], fp16, tag="xk1", name="xk1")
        xk2 = xp.tile([128, N_DT, 2 * NQ], fp16, tag="xk2", name="xk2")
        xq0 = xp.tile([128, N_DT, NQ], fp16, tag="xq0", name="xq0")
        xq1 = xp.tile([128, N_DT, NQ], fp16, tag="xq1", name="xq1")
        xq2 = xp.tile([128, N_DT, 2 * NQ], fp16, tag="xq2", name="xq2")
        # vT groups stay resident: read again by the pair-1 V projection
        xv = [xp.tile([128, N_DT, NQ], fp16, tag=f"xv{g}", name=f"xv{g}")
              for g in range(4)]
        # single queue, exact consumption order (deterministic arbitration)
        nc.sync.dma_start(wk_sb[:], wk.rearrange("(t p) e -> p t e", p=128))
        nc.sync.dma_start(xk0[:], kTr[:, :, 0:NQ])
        nc.sync.dma_start(wq_sb[:], wq.rearrange("(t p) e -> p t e", p=128))
        nc.sync.dma_start(bk_sb[:], bk.rearrange("(m p) o -> p (m o)", p=128))
        nc.sync.dma_start(bq_sb[:], bq.rearrange("(m p) o -> p (m o)", p=128))
        nc.sync.dma_start(xq0[:], qTr[:, :, 0:NQ])
        nc.sync.dma_start(xk1[:], kTr[:, :, NQ:2 * NQ])
        nc.sync.dma_start(xk2[:], kTr[:, :, 2 * NQ:S])
        nc.sync.dma_start(xq1[:], qTr[:, :, NQ:2 * NQ])
        nc.sync.dma_start(wv_sb[:], wv.rearrange("(t p) e -> p t e", p=128))
        for g in range(4):
            nc.sync.dma_start(xv[g][:], vTr[:, :, g * NQ:(g + 1) * NQ])
        nc.sync.dma_start(wo_sb[:], wo.rearrange("(t p) e -> p t e", p=128))
        nc.sync.dma_start(xq2[:], qTr[:, :, 2 * NQ:S])

        def k_rhs(n):
            if n == 0:
                return lambda d: xk0[:, d, :]
            if n == 1:
                return lambda d: xk1[:, d, :]
            return lambda d: xk2[:, d, (n - 2) * NQ:(n - 1) * NQ]

        def q_rhs(n):
            if n == 0:
                return lambda d: xq0[:, d, :]
            if n == 1:
                return lambda d: xq1[:, d, :]
            return lambda d: xq2[:, d, (n - 2) * NQ:(n - 1) * NQ]

        def proj_round(w_sb, b_sb, dst, m, n, rhs_of):
            # one (m, n) psum round of a [e, s] projection
            zb = aux.tile([128, NQ], f32, tag="z", name="pz")
            for d in range(N_DT):
                nc.tensor.matmul(zb[:], w_sb[:, d, m * 128:(m + 1) * 128],
                                 rhs_of(d), start=(d == 0),
                                 stop=(d == N_DT - 1))
            nc.vector.tensor_scalar_add(
                dst[m][:, n * NQ:(n + 1) * NQ], zb[:], b_sb[:, m:m + 1])

        def v_proj_stile(p, st):
            # V projection s-tile st, heads pair p -> vh[:, st, 2p:2p+2, :64]
            g, r = st // 4, st % 4
            vb = aux.tile([128, 2, DK], f32, tag="z", name="vb")
            for d in range(N_DT):
                nc.tensor.matmul(vb[:], xv[g][:, d, r * 128:(r + 1) * 128],
                                 wv_sb[:, d, p * 128:(p + 1) * 128],
                                 start=(d == 0), stop=(d == N_DT - 1))
            nc.vector.tensor_copy(vh[:, st, 2 * p:2 * p + 2, 0:DK], vb[:])

        def out_proj_pair(qc, otT, e2):
            # out_proj for e-tiles 2*e2, 2*e2+1 of q-chunk qc
            q0, q1 = qc * NQ, (qc + 1) * NQ
            zs = otp.tile([128, 2, NQ], fp16, tag="zs", name="zs")
            for c in range(2):
                et = 2 * e2 + c
                zb = aux.tile([128, NQ], f32, tag="z", name="zb")
                nc.tensor.matmul(zb[:], wo_sb[:, 0, et * 128:(et + 1) * 128],
                                 otT[:, 0, :], start=True, stop=False)
                nc.tensor.matmul(zb[:], wo_sb[:, 1, et * 128:(et + 1) * 128],
                                 otT[:, 1, :], start=False, stop=True)
                nc.vector.tensor_copy(zs[:, c, :], zb[:])
            nc.sync.dma_start(zTv[:, 2 * e2:2 * e2 + 2, q0:q1], zs[:])

        # preload the exp table set while the first DMAs stream
        dummy = persist.tile([1, 1], fp16, tag="dummy", name="dummy")
        nc.scalar.activation(dummy[:], ones1[:1, 0:1], AF.Exp)

        def pe_warm(n):
            # keep the PE busy-run alive across DMA-gated gaps so real
            # matmuls are charged at the warm p-state
            for _ in range(n):
                tw = aux.tile([128, 128], fp16, tag="z", name="tw")
                nc.tensor.matmul(tw[:], ident[:], ident[:], is_transpose=True,
                                 start=True, stop=True, skip_group_check=True)

        # prologue: pair-0 K/Q chunk 0 (gates the first scores); K n1+ via
        # chores so a late k1 DMA cannot block Q n0 in the PE FIFO
        pe_warm(42)
        proj_round(wk_sb, bk_sb, kh, 0, 0, k_rhs(0))
        pe_warm(12)
        proj_round(wq_sb, bq_sb, qh, 0, 0, q_rhs(0))

        # chore schedules: {slot: [callable, ...]} per (pair, qc) pass
        def v_chores(p, base, per_slot=2):
            d = {}
            for i in range(0, N_KT, per_slot):
                d.setdefault(base + i // per_slot, []).extend(
                    (lambda pp=p, st=st: v_proj_stile(pp, st))
                    for st in range(i, i + per_slot))
            return d

        ots = [otp.tile([128, 4, E], fp16, tag="ot", name=f"ot{qc}")
               for qc in range(N_QC)]
        otTs = [otp.tile([128, 2, NQ], fp16, tag="otT", name=f"otT{qc}")
                for qc in range(N_QC)]
        vh_ready = [0] * N_KT  # slot (in current pass) after which vh is valid
        prev_norm = None       # deferred (normalize, transpose) of prior pass

        for pas in range(8):
            p, qc = pas // 4, pas % 4
            q0, q1 = qc * NQ, (qc + 1) * NQ
            ot, otT = ots[qc], otTs[qc]
            chores = {}

            def add(slot, fn):
                chores.setdefault(slot, []).append(fn)

            def kc(m, n, slot):
                add(slot, lambda: proj_round(wk_sb, bk_sb, kh, m, n, k_rhs(n)))

            def qch(m, n, slot):
                add(slot, lambda: proj_round(wq_sb, bq_sb, qh, m, n, q_rhs(n)))

            if pas == 0:
                kc(0, 1, 2)
                kc(0, 2, 5)
                kc(0, 3, 7)
                qch(0, 1, 13)
                for s, l in v_chores(0, base=8, per_slot=2).items():
                    chores.setdefault(s, []).extend(l)
                for st in range(N_KT):
                    vh_ready[st] = 8 + st // 2
            elif pas == 4:
                qch(1, 1, 16)
                vslots = [2, 3, 4, 5, 6, 7, 7, 8, 8, 9, 10, 11, 12, 13, 14, 15]
                for st in range(N_KT):
                    add(vslots[st], lambda st=st: v_proj_stile(1, st))
                    vh_ready[st] = vslots[st]
            else:
                vh_ready = [-1] * N_KT
            if pas == 1:
                qch(0, 2, 4)
            elif pas == 2:
                qch(0, 3, 4)
                kc(1, 0, 9)
                kc(1, 1, 16)
            elif pas == 3:
                kc(1, 2, 7)
                kc(1, 3, 10)
                qch(1, 0, 16)
            elif pas >= 5:
                # out_proj of the previous q-chunk (both head pairs done);
                # on the final pass keep slot 16 clear to shorten the tail
                pqc = qc - 1
                for e2 in range(3):
                    add(3 + e2,
                        lambda pq=pqc, e=e2: out_proj_pair(pq, otTs[pq], e))
                add(13 if pas == 7 else 16,
                    lambda pq=pqc: out_proj_pair(pq, otTs[pq], 3))
                if qc < 3:
                    qch(1, qc + 1, 10)

            if prev_norm is not None:
                # previous pass's normalize+transpose runs in our first two
                # slots so its scores/exp never stall at the boundary
                add(0, prev_norm[0])
                add(1, prev_norm[1])
                prev_norm = None

            accs = [accp.tile([128, 4, DK + 1], f32, tag="acc",
                              name=f"acc{j}") for j in range(2)]
            pend = []   # kts whose P@V is not yet emitted

            def emit_pv(kt, pt):
                for j in range(2):
                    h = 2 * p + j
                    for qsb in range(4):
                        # start=True clears the whole bank row per written
                        # partition, so only the FIRST group in each acc bank
                        # may set it; later groups land on the cleared row
                        # (has_written=0 -> overwrite) and then accumulate.
                        nc.tensor.matmul(
                            accs[j][:, qsb, :],
                            pt[:, j * NQ + qsb * 128:j * NQ + (qsb + 1) * 128],
                            vh[:, kt, h, :], start=(kt == 0 and qsb == 0),
                            stop=(kt == N_KT - 1), skip_group_check=True)

            for kt in range(N_KT):
                sc = scp.tile([128, 2 * NQ], f32, tag="sc", name="sc")
                for j in range(2):
                    lo, hi = j * DK, (j + 1) * DK
                    nc.tensor.matmul(
                        sc[:, j * NQ:(j + 1) * NQ],
                        kh[p][lo:hi, kt * 128:(kt + 1) * 128],
                        qh[p][lo:hi, q0:q1], start=True, stop=True)
                pt = ptp.tile([128, 2 * NQ], fp16, tag="pt", name="pt")
                nc.scalar.activation(pt[:], sc[:], AF.Exp, scale=0.125)
                if _DEBUG and pas == 0 and kt in (0, 5, 10, 15):
                    dbg_pt = persist.tile([128, 2 * NQ], fp16,
                                          tag=f"dbgpt{kt}", name="dbgpt")
                    nc.vector.tensor_copy(dbg_pt[:], pt[:])
                    nc.sync.dma_start(dbg[f"pt{kt:02d}"][:, :], dbg_pt[:])
                for fn in chores.get(kt, ()):
                    fn()
                pend.append((kt, pt))
                # emit P@V for kts at least 1 slot old with vh available
                while pend and pend[0][0] < kt and vh_ready[pend[0][0]] <= kt:
                    emit_pv(*pend.pop(0))
            for fn in chores.get(16, ()):
                fn()
            for kt_, pt_ in pend:
                emit_pv(kt_, pt_)

            if _DEBUG and pas == 0:
                dbg_acc = persist.tile([128, 4 * (DK + 1)], f32, tag="dbgacc",
                                       name="dbgacc")
                nc.vector.tensor_copy(
                    dbg_acc[:].rearrange("p (a b) -> p a b", a=4),
                    accs[0][:, :, :])
                nc.sync.dma_start(dbg["acc0"][:, :], dbg_acc[:])

            def norm_half(j, accs=accs, p=p, ot=ot):
                # ot[q, e] = acc[:, :, 0:64] * (1 / acc[:, :, 64])
                h = 2 * p + j
                rv = otp.tile([128, 4], f32, tag="rv", name="rv")
                nc.vector.reciprocal(rv[:], accs[j][:, :, DK])
                for qsb in range(4):
                    nc.vector.tensor_scalar_mul(
                        ot[:, qsb, h * DK:(h + 1) * DK],
                        accs[j][:, qsb, 0:DK], rv[:, qsb:qsb + 1])

            def transp(accs=accs, p=p, ot=ot, otT=otT):
                # this pair's half of ot -> otT on the PE (4 blocks, 1 bank)
                tp = aux.tile([128, 4 * 128], fp16, tag="z", name="tp")
                for qsb in range(4):
                    nc.tensor.matmul(tp[:, qsb * 128:(qsb + 1) * 128],
                                     ot[:, qsb, p * 128:(p + 1) * 128],
                                     ident[:], is_transpose=True, start=True,
                                     stop=True, skip_group_check=True)
                nc.vector.tensor_copy(otT[:, p, :], tp[:])

            prev_norm = (lambda nh=norm_half: (nh(0), nh(1)),
                         lambda t=transp: t())

        # last pass's normalize + out_proj of the last q-chunk
        prev_norm[0]()
        prev_norm[1]()
        for e2 in range(4):
            out_proj_pair(N_QC - 1, otTs[N_QC - 1], e2)

        if _DEBUG:
            nc.sync.dma_start(dbg["kh0"][:, :], kh[0][:])
            nc.sync.dma_start(dbg["qh0"][:, :], qh[0][:])
            nc.sync.dma_start(
                dbg["vh"].rearrange("p (a b c) -> p a b c", b=HPC, c=DK + 1),
                vh[:, :, :, :])
            nc.sync.dma_start(
                dbg["ot0"].rearrange("p (a b) -> p a b", a=4), ots[0][:, :, :])
            nc.sync.dma_start(
                dbg["otT0"].rearrange("p (a b) -> p a b", a=2),
                otTs[0][:, :, :])

    nc.compile()
    return nc


def _get_program():
    global _PROGRAM
    if _PROGRAM is None:
        _PROGRAM = _build_program()
    return _PROGRAM


def _make_in_maps(q, k, v, Wq, bq, Wk, bk, Wv, Wo):
    f32 = np.float32
    f16 = np.float16
    xT = {}
    for b in range(B):
        xT[("q", b)] = np.ascontiguousarray(q[b].T, dtype=f16)
        xT[("k", b)] = np.ascontiguousarray(k[b].T, dtype=f16)
        xT[("v", b)] = np.ascontiguousarray(v[b].T, dtype=f16)
    ws = {}
    for g in range(4):
        sl = slice(g * E, (g + 1) * E)
        ws[("wq", g)] = np.ascontiguousarray(Wq[sl, :].T, dtype=f16)
        ws[("wk", g)] = np.ascontiguousarray(Wk[sl, :].T, dtype=f16)
        ws[("wv", g)] = np.ascontiguousarray(Wv[sl, :].T, dtype=f16)
        ws[("wo", g)] = np.ascontiguousarray(Wo[:, sl].T, dtype=f16)
        ws[("bq", g)] = np.ascontiguousarray(bq[sl].reshape(E, 1), dtype=f32)
        ws[("bk", g)] = np.ascontiguousarray(bk[sl].reshape(E, 1), dtype=f32)
    in_maps = []
    for c in range(N_CORES):
        b, g = c // 4, c % 4
        in_maps.append({
            "qT": xT[("q", b)], "kT": xT[("k", b)], "vT": xT[("v", b)],
            "wq": ws[("wq", g)], "wk": ws[("wk", g)], "wv": ws[("wv", g)],
            "wo": ws[("wo", g)], "bq": ws[("bq", g)], "bk": ws[("bk", g)],
        })
    return in_maps


def _numpy_fallback(q, k, v, mask, Wq, bq, Wk, bk, Wv, bv, Wo, bo):
    # Only used if mask is not all-True (never the case for this problem).
    def proj(x, W, b_):
        y = x @ W.T + b_
        return y.reshape(B, S, NUM_HEADS, DK).transpose(0, 2, 1, 3)
    qh, kh, vh = proj(q, Wq, bq), proj(k, Wk, bk), proj(v, Wv, bv)
    sc = np.einsum("bhqd,bhkd->bhqk", qh, kh) / np.sqrt(DK)
    sc = np.where(mask, sc, np.float32(-1e9))
    sc = sc - sc.max(-1, keepdims=True)
    p = np.exp(sc)
    p /= p.sum(-1, keepdims=True)
    o = np.einsum("bhqk,bhkd->bhqd", p, vh)
    o = o.transpose(0, 2, 1, 3).reshape(B, S, D_MODEL)
    return (o @ Wo.T + bo).astype(np.float32)


def kernel(q, k, v, mask, Wq, bq, Wk, bk, Wv, bv, Wo, bo):
    q = np.asarray(q, dtype=np.float32)
    k = np.asarray(k, dtype=np.float32)
    v = np.asarray(v, dtype=np.float32)
    Wq, Wk, Wv, Wo = (np.asarray(w, dtype=np.float32) for w in (Wq, Wk, Wv, Wo))
    bq, bk, bv, bo = (np.asarray(x, dtype=np.float32) for x in (bq, bk, bv, bo))
    if not np.all(np.asarray(mask)):
        return _numpy_fallback(q, k, v, np.asarray(mask), Wq, bq, Wk, bk,
                               Wv, bv, Wo, bo)

    from concourse.bass_utils import run_bass_kernel_spmd
    nc = _get_program()
    in_maps = _make_in_maps(q, k, v, Wq, bq, Wk, bk, Wv, Wo)
    res = run_bass_kernel_spmd(nc, in_maps, core_ids=list(range(N_CORES)),
                               **_RUN_KWARGS)
    global _LAST_RESULTS
    _LAST_RESULTS = res
    # V-bias folds out exactly: softmax rows sum to 1, so it contributes
    # bv @ Wo.T to every output row (added host-side with bo).
    bias_row = bo + bv @ Wo.T
    out = np.empty((B, S, D_MODEL), dtype=np.float32)
    for b in range(B):
        acc = res.results[4 * b]["zT"].astype(np.float32)
        for g in range(1, 4):
            acc = acc + res.results[4 * b + g]["zT"].astype(np.float32)
        out[b] = acc.T + bias_row
    return out


# revision 48
# speedup vs baseline: 1.5635x; 1.0065x over previous
"""Trainium2 Bass kernel for nn_MultiHeadAttention (B=2, S=2048, d_model=1024, H=16).

Sharding (8 cores): data-parallel over B (2) x tensor-parallel over head groups
(4 groups of 4 heads).  Each core computes its head-group's Q/K/V projections
(column-sharded weights), attention for its 4 heads, and a row-parallel
out_proj partial product.  The host sums the 4 partials per batch (the
"all-reduce") and adds the output bias.

Cost-model-aware layout (PE time = out-free-size x cycle; K, M are free):
  - Q/K projections land transposed [e, s]; V lands direct [s, e] with its
    bias applied by a rank-1 ones matmul and a constant ones column appended
    per head.
  - scores are [k, q] per 2-head pair into a 2-bank PSUM tile; one exp call
    covers 1024 columns.
  - P@V runs output-small: out [q=128, 65] per (head, q-subtile); column 64
    (against the ones column of V) accumulates the softmax denominator free.
  - normalization is per-partition reciprocal+scale on DVE; out tiles are
    transposed for out_proj by the XBAR dma-transpose; out_proj emits [d, q]
    fp16 partials.
  - engines execute in-order, so the outer iteration is head-pair-major
    (pair 0 for all q-chunks, then pair 1) and projection / out_proj /
    V-projection work is drip-fed into the ACT-bound k-loops via chore slots
    tuned to DMA arrival times.
"""

import sys
import numpy as np

for _p in ("/opt/trn_rl_repo", "/root/.axon_site/_ro/trn_rl_repo"):
    if _p not in sys.path:
        sys.path.append(_p)

D_MODEL = 1024
NUM_HEADS = 16
DK = 64
B = 2
S = 2048
N_CORES = 8
HPC = 4               # heads per core
E = HPC * DK          # 256 features per core
NQ = 512              # q-chunk size
N_QC = S // NQ        # 4 q chunks
N_KT = S // 128       # 16 k tiles
N_DT = D_MODEL // 128  # 8 contraction tiles for projections

_DEBUG = False
_PROGRAM = None
_RUN_KWARGS = {}      # test harness may set {"trace": True}
_LAST_RESULTS = None  # BassKernelResults of the last run


def _build_program():
    import concourse.mybir as mybir
    from concourse import bacc, tile
    from contextlib import ExitStack

    f32 = mybir.dt.float32
    fp16 = mybir.dt.float16
    AF = mybir.ActivationFunctionType

    nc = bacc.Bacc("TRN2", target_bir_lowering=False, debug=False,
                   num_devices=N_CORES)

    qT = nc.dram_tensor("qT", [D_MODEL, S], fp16, kind="ExternalInput").ap()
    kT = nc.dram_tensor("kT", [D_MODEL, S], fp16, kind="ExternalInput").ap()
    vT = nc.dram_tensor("vT", [D_MODEL, S], fp16, kind="ExternalInput").ap()
    wq = nc.dram_tensor("wq", [D_MODEL, E], fp16, kind="ExternalInput").ap()
    wk = nc.dram_tensor("wk", [D_MODEL, E], fp16, kind="ExternalInput").ap()
    wv = nc.dram_tensor("wv", [D_MODEL, E], fp16, kind="ExternalInput").ap()
    wo = nc.dram_tensor("wo", [E, D_MODEL], fp16, kind="ExternalInput").ap()
    bq = nc.dram_tensor("bq", [E, 1], f32, kind="ExternalInput").ap()
    bk = nc.dram_tensor("bk", [E, 1], f32, kind="ExternalInput").ap()
    zT = nc.dram_tensor("zT", [D_MODEL, S], fp16, kind="ExternalOutput").ap()
    dbg = None
    if _DEBUG:
        dbg = {
            "kh0": nc.dram_tensor("dkh0", [128, S], fp16,
                                  kind="ExternalOutput").ap(),
            "qh0": nc.dram_tensor("dqh0", [128, S], fp16,
                                  kind="ExternalOutput").ap(),
            "vh": nc.dram_tensor("dvh", [128, N_KT * HPC * (DK + 1)], fp16,
                                 kind="ExternalOutput").ap(),
            "ot0": nc.dram_tensor("dot0", [128, 4 * E], fp16,
                                  kind="ExternalOutput").ap(),
            "otT0": nc.dram_tensor("dotT0", [128, 2 * NQ], fp16,
                                   kind="ExternalOutput").ap(),
            "pt00": nc.dram_tensor("dpt00", [128, 2 * NQ], fp16,
                                   kind="ExternalOutput").ap(),
            "pt05": nc.dram_tensor("dpt05", [128, 2 * NQ], fp16,
                                   kind="ExternalOutput").ap(),
            "pt10": nc.dram_tensor("dpt10", [128, 2 * NQ], fp16,
                                   kind="ExternalOutput").ap(),
            "pt15": nc.dram_tensor("dpt15", [128, 2 * NQ], fp16,
                                   kind="ExternalOutput").ap(),
            "acc0": nc.dram_tensor("dacc0", [128, 4 * (DK + 1)], f32,
                                   kind="ExternalOutput").ap(),
        }
    zTv = zT.rearrange("(a p) q -> p a q", p=128)  # [128, 8, 2048]

    with tile.TileContext(nc) as tc, ExitStack() as ctx:
        persist = ctx.enter_context(tc.tile_pool(name="persist", bufs=1))
        xp = ctx.enter_context(tc.tile_pool(name="xp", bufs=1))
        ptp = ctx.enter_context(tc.tile_pool(name="ptp", bufs=10))
        otp = ctx.enter_context(tc.tile_pool(name="otp", bufs=4))
        scp = ctx.enter_context(tc.tile_pool(name="scp", bufs=2, space="PSUM"))
        accp = ctx.enter_context(tc.tile_pool(name="accp", bufs=2,
                                              space="PSUM"))
        aux = ctx.enter_context(tc.tile_pool(name="aux", bufs=2, space="PSUM"))

        # ---- persistent SBUF tensors ----------------------------------
        wq_sb = persist.tile([128, N_DT, E], fp16, tag="wq", name="wq")
        wk_sb = persist.tile([128, N_DT, E], fp16, tag="wk", name="wk")
        wv_sb = persist.tile([128, N_DT, E], fp16, tag="wv", name="wv")
        wo_sb = persist.tile([128, 2, D_MODEL], fp16, tag="wo", name="wo")
        bq_sb = persist.tile([128, 2], f32, tag="bq", name="bq")
        bk_sb = persist.tile([128, 2], f32, tag="bk", name="bk")
        ones1 = persist.tile([1, 128], fp16, tag="ones1", name="ones1")
        kh = [persist.tile([128, S], fp16, tag=f"kh{p}", name=f"kh{p}")
              for p in range(2)]
        qh = [persist.tile([128, S], fp16, tag=f"qh{p}", name=f"qh{p}")
              for p in range(2)]
        # V projection [s-tile, head, dk+1]; col 64 is the constant 1.0
        vh = persist.tile([128, N_KT, HPC, DK + 1], fp16, tag="vh", name="vh")

        nc.vector.memset(ones1[:], 1.0)
        nc.vector.memset(vh[:, :, :, DK], 1.0)
        from concourse.masks import make_identity
        ident = persist.tile([128, 128], fp16, tag="ident", name="ident")
        make_identity(nc, ident)

        # ---- input streaming (sync/SP queue, in consumption order) ----
        # one DMA per 512-column group: the SP sequencer costs ~650ns per
        # DMA, so grouped [128, d, cols] transfers beat per-d-tile chunks
        kTr = kT.rearrange("(t p) q -> p t q", p=128)
        qTr = qT.rearrange("(t p) q -> p t q", p=128)
        vTr = vT.rearrange("(t p) q -> p t q", p=128)
        xk0 = xp.tile([128, N_DT, NQ], fp16, tag="xk0", name="xk0")
        xk1 = xp.tile([128, N_DT, NQ], fp16, tag="xk1", name="xk1")
        xk2 = xp.tile([128, N_DT, 2 * NQ], fp16, tag="xk2", name="xk2")
        xk2a, xk2b = xk2[:, :, 0:NQ], xk2[:, :, NQ:2 * NQ]
        xq0 = xp.tile([128, N_DT, NQ], fp16, tag="xq0", name="xq0")
        xq1 = xp.tile([128, N_DT, NQ], fp16, tag="xq1", name="xq1")
        xq2 = xp.tile([128, N_DT, 2 * NQ], fp16, tag="xq2", name="xq2")
        xq2a, xq2b = xq2[:, :, 0:NQ], xq2[:, :, NQ:2 * NQ]
        # vT groups stay resident: read again by the pair-1 V projection
        xv = [xp.tile([128, N_DT, NQ], fp16, tag=f"xv{g}", name=f"xv{g}")
              for g in range(4)]
        # single queue, exact consumption order (deterministic arbitration)
        nc.sync.dma_start(wk_sb[:], wk.rearrange("(t p) e -> p t e", p=128))
        nc.sync.dma_start(xk0[:], kTr[:, :, 0:NQ])
        nc.sync.dma_start(wq_sb[:], wq.rearrange("(t p) e -> p t e", p=128))
        nc.sync.dma_start(bk_sb[:], bk.rearrange("(m p) o -> p (m o)", p=128))
        nc.sync.dma_start(bq_sb[:], bq.rearrange("(m p) o -> p (m o)", p=128))
        nc.sync.dma_start(xq0[:], qTr[:, :, 0:NQ])
        nc.sync.dma_start(xk1[:], kTr[:, :, NQ:2 * NQ])
        nc.sync.dma_start(wv_sb[:], wv.rearrange("(t p) e -> p t e", p=128))
        nc.sync.dma_start(xv[0][:], vTr[:, :, 0:NQ])
        nc.sync.dma_start(xk2a, kTr[:, :, 2 * NQ:3 * NQ])
        nc.sync.dma_start(xv[1][:], vTr[:, :, NQ:2 * NQ])
        nc.sync.dma_start(xk2b, kTr[:, :, 3 * NQ:S])
        nc.sync.dma_start(xq1[:], qTr[:, :, NQ:2 * NQ])
        nc.sync.dma_start(xv[2][:], vTr[:, :, 2 * NQ:3 * NQ])
        nc.sync.dma_start(xv[3][:], vTr[:, :, 3 * NQ:S])
        nc.sync.dma_start(wo_sb[:], wo.rearrange("(t p) e -> p t e", p=128))
        nc.sync.dma_start(xq2a, qTr[:, :, 2 * NQ:3 * NQ])
        nc.sync.dma_start(xq2b, qTr[:, :, 3 * NQ:S])

        def k_rhs(n):
            if n == 0:
                return lambda d: xk0[:, d, :]
            if n == 1:
                return lambda d: xk1[:, d, :]
            return lambda d: xk2[:, d, (n - 2) * NQ:(n - 1) * NQ]

        def q_rhs(n):
            if n == 0:
                return lambda d: xq0[:, d, :]
            if n == 1:
                return lambda d: xq1[:, d, :]
            return lambda d: xq2[:, d, (n - 2) * NQ:(n - 1) * NQ]

        def proj_half(w_sb, b_sb, dst, m, n, rhs_of, half, cell):
            # half-round of a [e, s] projection (split for chore smoothing)
            if half == 0:
                cell.append(aux.tile([128, NQ], f32, tag="z", name="pz"))
            zb = cell[0]
            for d in (range(4) if half == 0 else range(4, N_DT)):
                nc.tensor.matmul(zb[:], w_sb[:, d, m * 128:(m + 1) * 128],
                                 rhs_of(d), start=(d == 0),
                                 stop=(d == N_DT - 1))
            if half == 1:
                nc.vector.tensor_scalar_add(
                    dst[m][:, n * NQ:(n + 1) * NQ], zb[:], b_sb[:, m:m + 1])

        def v_half(p, st, half, cell):
            # half of a V-projection s-tile -> vh[:, st, 2p:2p+2, :64]
            g, r = st // 4, st % 4
            if half == 0:
                cell.append(aux.tile([128, 2, DK], f32, tag="z", name="vb"))
            vb = cell[0]
            for d in (range(4) if half == 0 else range(4, N_DT)):
                nc.tensor.matmul(vb[:], xv[g][:, d, r * 128:(r + 1) * 128],
                                 wv_sb[:, d, p * 128:(p + 1) * 128],
                                 start=(d == 0), stop=(d == N_DT - 1))
            if half == 1:
                nc.vector.tensor_copy(vh[:, st, 2 * p:2 * p + 2, 0:DK], vb[:])

        def proj_round(w_sb, b_sb, dst, m, n, rhs_of):
            cell = []
            proj_half(w_sb, b_sb, dst, m, n, rhs_of, 0, cell)
            proj_half(w_sb, b_sb, dst, m, n, rhs_of, 1, cell)

        def kq_job(which, m, n, earliest):
            w_sb, b_sb, dst, rhs = ((wk_sb, bk_sb, kh, k_rhs(n))
                                    if which == "k" else
                                    (wq_sb, bq_sb, qh, q_rhs(n)))
            cell = []
            return {"earliest": earliest, "w": [0.89, 0.93], "atoms": [
                lambda h=0: proj_half(w_sb, b_sb, dst, m, n, rhs, 0, cell),
                lambda h=1: proj_half(w_sb, b_sb, dst, m, n, rhs, 1, cell)]}

        def v_job(p, st, earliest):
            cell = []
            return {"earliest": earliest, "w": [0.46, 0.5], "v_st": st,
                    "atoms": [
                        lambda: v_half(p, st, 0, cell),
                        lambda: v_half(p, st, 1, cell)]}

        def op_job(pqc, e2, earliest):
            cell = []
            return {"earliest": earliest, "w": [0.55, 0.55], "atoms": [
                lambda c=0: out_proj_c(pqc, otTs[pqc], e2, c, cell),
                lambda c=1: out_proj_c(pqc, otTs[pqc], e2, c, cell)]}

        def schedule(jobs):
            """Greedy per-slot packing of 2-atom jobs: at most two jobs'
            psum ring tiles open at once, atoms in order, per-slot budget."""
            BUDGET = 1.00
            chores, vh_rd = {}, {}
            open_q = []
            waiting = list(jobs)
            for slot in range(17):
                cap = 1e9 if slot == 16 else (BUDGET if slot != 1 else 1.0)
                while True:
                    si = next((i for i, j in enumerate(waiting)
                               if j["earliest"] <= slot or slot == 16), None)
                    if open_q and (len(open_q) >= 2 or si is None):
                        job, ai = open_q[0]
                        w = job["w"][ai]
                        if w > cap:
                            break
                        chores.setdefault(slot, []).append(job["atoms"][ai])
                        cap -= w
                        if ai + 1 == len(job["atoms"]):
                            open_q.pop(0)
                            if "v_st" in job:
                                vh_rd[job["v_st"]] = slot
                        else:
                            open_q[0] = (job, ai + 1)
                    elif si is not None:
                        job = waiting[si]
                        w = job["w"][0]
                        if w > cap:
                            break
                        waiting.pop(si)
                        chores.setdefault(slot, []).append(job["atoms"][0])
                        cap -= w
                        open_q.append((job, 1))
                    else:
                        break
            assert not open_q and not waiting, "chore overflow"
            return chores, vh_rd

        def out_proj_c(qc, otT, e2, c, cell):
            # one e-tile column (c) of out_proj for e-pair e2, q-chunk qc
            q0, q1 = qc * NQ, (qc + 1) * NQ
            if c == 0:
                cell.append(otp.tile([128, 2, NQ], fp16, tag="zs", name="zs",
                                     bufs=8))
            zs = cell[0]
            et = 2 * e2 + c
            zb = aux.tile([128, NQ], f32, tag="z", name="zb")
            nc.tensor.matmul(zb[:], wo_sb[:, 0, et * 128:(et + 1) * 128],
                             otT[:, 0, :], start=True, stop=False)
            nc.tensor.matmul(zb[:], wo_sb[:, 1, et * 128:(et + 1) * 128],
                             otT[:, 1, :], start=False, stop=True)
            nc.vector.tensor_copy(zs[:, c, :], zb[:])
            if c == 1:
                nc.sync.dma_start(zTv[:, 2 * e2:2 * e2 + 2, q0:q1], zs[:])

        def out_proj_tail(qc, otT, e2):
            # tail variant: zb pairs alternate between the free scores pool
            # and the aux pool (6 banks total), second copy on the idle ACT
            q0, q1 = qc * NQ, (qc + 1) * NQ
            zs = otp.tile([128, 2, NQ], fp16, tag="zs", name="zs", bufs=8)
            if e2 % 2 == 0:
                zb2 = scp.tile([128, 2, NQ], f32, tag="sc", name="zb2")
                zbs = [zb2[:, 0, :], zb2[:, 1, :]]
            else:
                zbs = [aux.tile([128, NQ], f32, tag="z", name="zba"),
                       aux.tile([128, NQ], f32, tag="z", name="zbb")]
            for c in range(2):
                et = 2 * e2 + c
                nc.tensor.matmul(zbs[c], wo_sb[:, 0, et * 128:(et + 1) * 128],
                                 otT[:, 0, :], start=True, stop=False)
                nc.tensor.matmul(zbs[c], wo_sb[:, 1, et * 128:(et + 1) * 128],
                                 otT[:, 1, :], start=False, stop=True)
                if c == 1:
                    nc.scalar.activation(zs[:, c, :], zbs[c], AF.Copy)
                else:
                    nc.vector.tensor_copy(zs[:, c, :], zbs[c])
            nc.sync.dma_start(zTv[:, 2 * e2:2 * e2 + 2, q0:q1], zs[:])

        # preload the exp table set while the first DMAs stream
        dummy = persist.tile([1, 1], fp16, tag="dummy", name="dummy")
        nc.scalar.activation(dummy[:], ones1[:1, 0:1], AF.Exp)

        def pe_warm(n):
            # keep the PE busy-run alive across DMA-gated gaps so real
            # matmuls are charged at the warm p-state
            for _ in range(n):
                tw = aux.tile([128, 128], fp16, tag="z", name="tw")
                nc.tensor.matmul(tw[:], ident[:], ident[:], is_transpose=True,
                                 start=True, stop=True, skip_group_check=True)

        # prologue: pair-0 K/Q chunk 0 (gates the first scores); K n1+ via
        # chores so a late k1 DMA cannot block Q n0 in the PE FIFO
        pe_warm(42)
        proj_round(wk_sb, bk_sb, kh, 0, 0, k_rhs(0))
        pe_warm(12)
        proj_round(wq_sb, bq_sb, qh, 0, 0, q_rhs(0))

        ots = [otp.tile([128, 4, E], fp16, tag="ot", name=f"ot{qc}")
               for qc in range(N_QC)]
        otTs = [otp.tile([128, 2, NQ], fp16, tag="otT", name=f"otT{qc}")
                for qc in range(N_QC)]
        vh_ready = [0] * N_KT  # slot (in current pass) after which vh is valid
        prev_norm = None       # deferred (normalize, transpose) of prior pass

        for pas in range(8):
            p, qc = pas // 4, pas % 4
            q0, q1 = qc * NQ, (qc + 1) * NQ
            ot, otT = ots[qc], otTs[qc]

            # job lists in priority order (earliest slots from DMA arrivals)
            jobs = []
            if pas == 0:
                jobs.append(kq_job("k", 0, 1, 1))
                jobs.append(kq_job("k", 0, 2, 5))
                jobs.append(kq_job("k", 0, 3, 7))
                jobs += [v_job(0, st, 4) for st in range(4)]
                jobs += [v_job(0, st, 6) for st in range(4, 8)]
                jobs += [v_job(0, st, 10) for st in range(8, 12)]
                jobs.append(kq_job("q", 0, 1, 9))
                jobs += [v_job(0, st, 11) for st in range(12, 16)]
            elif pas == 1:
                jobs.append(kq_job("q", 0, 2, 2))
                jobs.append(kq_job("k", 1, 0, 2))
                jobs += [v_job(1, st, 2) for st in range(8)]
            elif pas == 2:
                jobs.append(kq_job("q", 0, 3, 2))
                jobs.append(kq_job("k", 1, 1, 2))
                jobs += [v_job(1, st, 2) for st in range(8, 16)]
            elif pas == 3:
                jobs.append(kq_job("k", 1, 2, 2))
                jobs.append(kq_job("k", 1, 3, 2))
                jobs.append(kq_job("q", 1, 0, 2))
            elif pas == 4:
                jobs.append(kq_job("q", 1, 1, 2))
                jobs.append(kq_job("q", 1, 2, 8))
            else:
                pqc = qc - 1
                jobs += [op_job(pqc, e2, 2) for e2 in range(4)]
                if qc == 1:
                    jobs.append(kq_job("q", 1, 3, 8))
            chores, vh_rd = schedule(jobs)
            if pas == 0:
                for st in range(N_KT):
                    vh_ready[st] = vh_rd[st]
            else:
                vh_ready = [-1] * N_KT

            if prev_norm is not None:
                # previous pass's normalize+transpose runs in our first two
                # slots so its scores/exp never stall at the boundary
                chores.setdefault(0, []).insert(0, prev_norm[0])
                chores.setdefault(1, []).insert(0, prev_norm[1])
                prev_norm = None

            accs = [accp.tile([128, 4, DK + 1], f32, tag="acc",
                              name=f"acc{j}") for j in range(2)]
            pend = []   # kts whose P@V is not yet emitted

            def emit_pv(kt, pt):
                for j in range(2):
                    h = 2 * p + j
                    for qsb in range(4):
                        # start=True clears the whole bank row per written
                        # partition, so only the FIRST group in each acc bank
                        # may set it; later groups land on the cleared row
                        # (has_written=0 -> overwrite) and then accumulate.
                        nc.tensor.matmul(
                            accs[j][:, qsb, :],
                            pt[:, j * NQ + qsb * 128:j * NQ + (qsb + 1) * 128],
                            vh[:, kt, h, :], start=(kt == 0 and qsb == 0),
                            stop=(kt == N_KT - 1), skip_group_check=True)

            for kt in range(N_KT):
                sc = scp.tile([128, 2 * NQ], f32, tag="sc", name="sc")
                for j in range(2):
                    lo, hi = j * DK, (j + 1) * DK
                    nc.tensor.matmul(
                        sc[:, j * NQ:(j + 1) * NQ],
                        kh[p][lo:hi, kt * 128:(kt + 1) * 128],
                        qh[p][lo:hi, q0:q1], start=True, stop=True)
                pt = ptp.tile([128, 2 * NQ], fp16, tag="pt", name="pt")
                nc.scalar.activation(pt[:], sc[:], AF.Exp, scale=0.125)
                if _DEBUG and pas == 0 and kt in (0, 5, 10, 15):
                    dbg_pt = persist.tile([128, 2 * NQ], fp16,
                                          tag=f"dbgpt{kt}", name="dbgpt")
                    nc.vector.tensor_copy(dbg_pt[:], pt[:])
                    nc.sync.dma_start(dbg[f"pt{kt:02d}"][:, :], dbg_pt[:])
                for fn in chores.get(kt, ()):
                    fn()
                pend.append((kt, pt))
                # emit P@V for kts at least 1 slot old with vh available
                while pend and pend[0][0] < kt and vh_ready[pend[0][0]] <= kt:
                    emit_pv(*pend.pop(0))
            for fn in chores.get(16, ()):
                fn()
            for kt_, pt_ in pend:
                emit_pv(kt_, pt_)
            leftover = []

            if _DEBUG and pas == 0:
                dbg_acc = persist.tile([128, 4 * (DK + 1)], f32, tag="dbgacc",
                                       name="dbgacc")
                nc.vector.tensor_copy(
                    dbg_acc[:].rearrange("p (a b) -> p a b", a=4),
                    accs[0][:, :, :])
                nc.sync.dma_start(dbg["acc0"][:, :], dbg_acc[:])

            def norm_half(j, accs=accs, p=p, ot=ot, on_act=False):
                # ot[q, e] = acc[:, :, 0:64] * (1 / acc[:, :, 64])
                h = 2 * p + j
                rv = otp.tile([128, 4], f32, tag="rv", name="rv")
                nc.vector.reciprocal(rv[:], accs[j][:, :, DK])
                for qsb in range(4):
                    if on_act:
                        nc.scalar.activation(
                            ot[:, qsb, h * DK:(h + 1) * DK],
                            accs[j][:, qsb, 0:DK], AF.Copy,
                            scale=rv[:, qsb:qsb + 1])
                    else:
                        nc.vector.tensor_scalar_mul(
                            ot[:, qsb, h * DK:(h + 1) * DK],
                            accs[j][:, qsb, 0:DK], rv[:, qsb:qsb + 1])

            def transp(accs=accs, p=p, ot=ot, otT=otT):
                # this pair's half of ot -> otT on the PE (4 blocks, 1 bank)
                tp = aux.tile([128, 4 * 128], fp16, tag="z", name="tp")
                for qsb in range(4):
                    nc.tensor.matmul(tp[:, qsb * 128:(qsb + 1) * 128],
                                     ot[:, qsb, p * 128:(p + 1) * 128],
                                     ident[:], is_transpose=True, start=True,
                                     stop=True, skip_group_check=True)
                nc.vector.tensor_copy(otT[:, p, :], tp[:])

            prev_norm = (lambda nh=norm_half: (nh(0), nh(1)),
                         lambda t=transp: t(), norm_half,
                         lambda lo=leftover, ep=emit_pv:
                             [ep(k_, p_) for k_, p_ in lo])

        # last pass's normalize (split across DVE+ACT) + final out_proj
        prev_norm[2](0)
        prev_norm[2](1, on_act=True)
        prev_norm[1]()
        for e2 in range(4):
            out_proj_tail(N_QC - 1, otTs[N_QC - 1], e2)

        if _DEBUG:
            nc.sync.dma_start(dbg["kh0"][:, :], kh[0][:])
            nc.sync.dma_start(dbg["qh0"][:, :], qh[0][:])
            nc.sync.dma_start(
                dbg["vh"].rearrange("p (a b c) -> p a b c", b=HPC, c=DK + 1),
                vh[:, :, :, :])
            nc.sync.dma_start(
                dbg["ot0"].rearrange("p (a b) -> p a b", a=4), ots[0][:, :, :])
            nc.sync.dma_start(
                dbg["otT0"].rearrange("p (a b) -> p a b", a=2),
                otTs[0][:, :, :])

    nc.compile()
    return nc


def _get_program():
    global _PROGRAM
    if _PROGRAM is None:
        _PROGRAM = _build_program()
    return _PROGRAM


def _make_in_maps(q, k, v, Wq, bq, Wk, bk, Wv, Wo):
    f32 = np.float32
    f16 = np.float16
    xT = {}
    for b in range(B):
        xT[("q", b)] = np.ascontiguousarray(q[b].T, dtype=f16)
        xT[("k", b)] = np.ascontiguousarray(k[b].T, dtype=f16)
        xT[("v", b)] = np.ascontiguousarray(v[b].T, dtype=f16)
    ws = {}
    for g in range(4):
        sl = slice(g * E, (g + 1) * E)
        ws[("wq", g)] = np.ascontiguousarray(Wq[sl, :].T, dtype=f16)
        ws[("wk", g)] = np.ascontiguousarray(Wk[sl, :].T, dtype=f16)
        ws[("wv", g)] = np.ascontiguousarray(Wv[sl, :].T, dtype=f16)
        ws[("wo", g)] = np.ascontiguousarray(Wo[:, sl].T, dtype=f16)
        ws[("bq", g)] = np.ascontiguousarray(bq[sl].reshape(E, 1), dtype=f32)
        ws[("bk", g)] = np.ascontiguousarray(bk[sl].reshape(E, 1), dtype=f32)
    in_maps = []
    for c in range(N_CORES):
        b, g = c // 4, c % 4
        in_maps.append({
            "qT": xT[("q", b)], "kT": xT[("k", b)], "vT": xT[("v", b)],
            "wq": ws[("wq", g)], "wk": ws[("wk", g)], "wv": ws[("wv", g)],
            "wo": ws[("wo", g)], "bq": ws[("bq", g)], "bk": ws[("bk", g)],
        })
    return in_maps


def _numpy_fallback(q, k, v, mask, Wq, bq, Wk, bk, Wv, bv, Wo, bo):
    # Only used if mask is not all-True (never the case for this problem).
    def proj(x, W, b_):
        y = x @ W.T + b_
        return y.reshape(B, S, NUM_HEADS, DK).transpose(0, 2, 1, 3)
    qh, kh, vh = proj(q, Wq, bq), proj(k, Wk, bk), proj(v, Wv, bv)
    sc = np.einsum("bhqd,bhkd->bhqk", qh, kh) / np.sqrt(DK)
    sc = np.where(mask, sc, np.float32(-1e9))
    sc = sc - sc.max(-1, keepdims=True)
    p = np.exp(sc)
    p /= p.sum(-1, keepdims=True)
    o = np.einsum("bhqk,bhkd->bhqd", p, vh)
    o = o.transpose(0, 2, 1, 3).reshape(B, S, D_MODEL)
    return (o @ Wo.T + bo).astype(np.float32)


def kernel(q, k, v, mask, Wq, bq, Wk, bk, Wv, bv, Wo, bo):
    q = np.asarray(q, dtype=np.float32)
    k = np.asarray(k, dtype=np.float32)
    v = np.asarray(v, dtype=np.float32)
    Wq, Wk, Wv, Wo = (np.asarray(w, dtype=np.float32) for w in (Wq, Wk, Wv, Wo))
    bq, bk, bv, bo = (np.asarray(x, dtype=np.float32) for x in (bq, bk, bv, bo))
    if not np.all(np.asarray(mask)):
        return _numpy_fallback(q, k, v, np.asarray(mask), Wq, bq, Wk, bk,
                               Wv, bv, Wo, bo)

    from concourse.bass_utils import run_bass_kernel_spmd
    nc = _get_program()
    in_maps = _make_in_maps(q, k, v, Wq, bq, Wk, bk, Wv, Wo)
    res = run_bass_kernel_spmd(nc, in_maps, core_ids=list(range(N_CORES)),
                               **_RUN_KWARGS)
    global _LAST_RESULTS
    _LAST_RESULTS = res
    # V-bias folds out exactly: softmax rows sum to 1, so it contributes
    # bv @ Wo.T to every output row (added host-side with bo).
    bias_row = bo + bv @ Wo.T
    out = np.empty((B, S, D_MODEL), dtype=np.float32)
    for b in range(B):
        acc = res.results[4 * b]["zT"].astype(np.float32)
        for g in range(1, 4):
            acc = acc + res.results[4 * b + g]["zT"].astype(np.float32)
        out[b] = acc.T + bias_row
    return out


# revision 50
# speedup vs baseline: 1.5835x; 1.0128x over previous
"""Trainium2 Bass kernel for nn_MultiHeadAttention (B=2, S=2048, d_model=1024, H=16).

Sharding (8 cores): data-parallel over B (2) x tensor-parallel over head groups
(4 groups of 4 heads).  Each core computes its head-group's Q/K/V projections
(column-sharded weights), attention for its 4 heads, and a row-parallel
out_proj partial product.  The host sums the 4 partials per batch (the
"all-reduce") and adds the output bias.

Cost-model-aware layout (PE time = out-free-size x cycle; K, M are free):
  - Q/K projections land transposed [e, s]; V lands direct [s, e] with its
    bias applied by a rank-1 ones matmul and a constant ones column appended
    per head.
  - scores are [k, q] per 2-head pair into a 2-bank PSUM tile; one exp call
    covers 1024 columns.
  - P@V runs output-small: out [q=128, 65] per (head, q-subtile); column 64
    (against the ones column of V) accumulates the softmax denominator free.
  - normalization is per-partition reciprocal+scale on DVE; out tiles are
    transposed for out_proj by the XBAR dma-transpose; out_proj emits [d, q]
    fp16 partials.
  - engines execute in-order, so the outer iteration is head-pair-major
    (pair 0 for all q-chunks, then pair 1) and projection / out_proj /
    V-projection work is drip-fed into the ACT-bound k-loops via chore slots
    tuned to DMA arrival times.
"""

import sys
import numpy as np

for _p in ("/opt/trn_rl_repo", "/root/.axon_site/_ro/trn_rl_repo"):
    if _p not in sys.path:
        sys.path.append(_p)

D_MODEL = 1024
NUM_HEADS = 16
DK = 64
B = 2
S = 2048
N_CORES = 8
HPC = 4               # heads per core
E = HPC * DK          # 256 features per core
NQ = 512              # q-chunk size
N_QC = S // NQ        # 4 q chunks
N_KT = S // 128       # 16 k tiles
N_DT = D_MODEL // 128  # 8 contraction tiles for projections

_DEBUG = False
_PROGRAM = None
_RUN_KWARGS = {}      # test harness may set {"trace": True}
_LAST_RESULTS = None  # BassKernelResults of the last run


def _build_program():
    import concourse.mybir as mybir
    from concourse import bacc, tile
    from contextlib import ExitStack

    f32 = mybir.dt.float32
    fp16 = mybir.dt.float16
    AF = mybir.ActivationFunctionType

    nc = bacc.Bacc("TRN2", target_bir_lowering=False, debug=False,
                   num_devices=N_CORES)

    qT = nc.dram_tensor("qT", [D_MODEL, S], fp16, kind="ExternalInput").ap()
    kT = nc.dram_tensor("kT", [D_MODEL, S], fp16, kind="ExternalInput").ap()
    vT = nc.dram_tensor("vT", [D_MODEL, S], fp16, kind="ExternalInput").ap()
    wq = nc.dram_tensor("wq", [D_MODEL, E], fp16, kind="ExternalInput").ap()
    wk = nc.dram_tensor("wk", [D_MODEL, E], fp16, kind="ExternalInput").ap()
    wv = nc.dram_tensor("wv", [D_MODEL, E], fp16, kind="ExternalInput").ap()
    wo = nc.dram_tensor("wo", [E, D_MODEL], fp16, kind="ExternalInput").ap()
    bq = nc.dram_tensor("bq", [E, 1], f32, kind="ExternalInput").ap()
    bk = nc.dram_tensor("bk", [E, 1], f32, kind="ExternalInput").ap()
    zT = nc.dram_tensor("zT", [D_MODEL, S], fp16, kind="ExternalOutput").ap()
    dbg = None
    if _DEBUG:
        dbg = {
            "kh0": nc.dram_tensor("dkh0", [128, S], fp16,
                                  kind="ExternalOutput").ap(),
            "qh0": nc.dram_tensor("dqh0", [128, S], fp16,
                                  kind="ExternalOutput").ap(),
            "vh": nc.dram_tensor("dvh", [128, N_KT * HPC * (DK + 1)], fp16,
                                 kind="ExternalOutput").ap(),
            "ot0": nc.dram_tensor("dot0", [128, 4 * E], fp16,
                                  kind="ExternalOutput").ap(),
            "otT0": nc.dram_tensor("dotT0", [128, 2 * NQ], fp16,
                                   kind="ExternalOutput").ap(),
            "pt00": nc.dram_tensor("dpt00", [128, 2 * NQ], fp16,
                                   kind="ExternalOutput").ap(),
            "pt05": nc.dram_tensor("dpt05", [128, 2 * NQ], fp16,
                                   kind="ExternalOutput").ap(),
            "pt10": nc.dram_tensor("dpt10", [128, 2 * NQ], fp16,
                                   kind="ExternalOutput").ap(),
            "pt15": nc.dram_tensor("dpt15", [128, 2 * NQ], fp16,
                                   kind="ExternalOutput").ap(),
            "acc0": nc.dram_tensor("dacc0", [128, 4 * (DK + 1)], f32,
                                   kind="ExternalOutput").ap(),
        }
    zTv = zT.rearrange("(a p) q -> p a q", p=128)  # [128, 8, 2048]

    with tile.TileContext(nc) as tc, ExitStack() as ctx:
        persist = ctx.enter_context(tc.tile_pool(name="persist", bufs=1))
        xp = ctx.enter_context(tc.tile_pool(name="xp", bufs=1))
        ptp = ctx.enter_context(tc.tile_pool(name="ptp", bufs=10))
        otp = ctx.enter_context(tc.tile_pool(name="otp", bufs=4))
        scp = ctx.enter_context(tc.tile_pool(name="scp", bufs=2, space="PSUM"))
        accp = ctx.enter_context(tc.tile_pool(name="accp", bufs=2,
                                              space="PSUM"))
        aux = ctx.enter_context(tc.tile_pool(name="aux", bufs=2, space="PSUM"))

        # ---- persistent SBUF tensors ----------------------------------
        wq_sb = persist.tile([128, N_DT, E], fp16, tag="wq", name="wq")
        wk_sb = persist.tile([128, N_DT, E], fp16, tag="wk", name="wk")
        wv_sb = persist.tile([128, N_DT, E], fp16, tag="wv", name="wv")
        wo_sb = persist.tile([128, 2, D_MODEL], fp16, tag="wo", name="wo")
        bq_sb = persist.tile([128, 2], f32, tag="bq", name="bq")
        bk_sb = persist.tile([128, 2], f32, tag="bk", name="bk")
        ones1 = persist.tile([1, 128], fp16, tag="ones1", name="ones1")
        kh = [persist.tile([128, S], fp16, tag=f"kh{p}", name=f"kh{p}")
              for p in range(2)]
        qh = [persist.tile([128, S], fp16, tag=f"qh{p}", name=f"qh{p}")
              for p in range(2)]
        # V projection [s-tile, head, dk+1]; col 64 is the constant 1.0
        vh = persist.tile([128, N_KT, HPC, DK + 1], fp16, tag="vh", name="vh")

        nc.vector.memset(ones1[:], 1.0)
        nc.vector.memset(vh[:, :, :, DK], 1.0)
        from concourse.masks import make_identity
        ident = persist.tile([128, 128], fp16, tag="ident", name="ident")
        make_identity(nc, ident)

        # ---- input streaming (sync/SP queue, in consumption order) ----
        # one DMA per 512-column group: the SP sequencer costs ~650ns per
        # DMA, so grouped [128, d, cols] transfers beat per-d-tile chunks
        kTr = kT.rearrange("(t p) q -> p t q", p=128)
        qTr = qT.rearrange("(t p) q -> p t q", p=128)
        vTr = vT.rearrange("(t p) q -> p t q", p=128)
        xk0 = xp.tile([128, N_DT, NQ], fp16, tag="xk0", name="xk0")
        xk1 = xp.tile([128, N_DT, NQ], fp16, tag="xk1", name="xk1")
        xk2 = xp.tile([128, N_DT, 2 * NQ], fp16, tag="xk2", name="xk2")
        xk2a, xk2b = xk2[:, :, 0:NQ], xk2[:, :, NQ:2 * NQ]
        xq0 = xp.tile([128, N_DT, NQ], fp16, tag="xq0", name="xq0")
        xq1 = xp.tile([128, N_DT, NQ], fp16, tag="xq1", name="xq1")
        xq2 = xp.tile([128, N_DT, 2 * NQ], fp16, tag="xq2", name="xq2")
        xq2a, xq2b = xq2[:, :, 0:NQ], xq2[:, :, NQ:2 * NQ]
        # vT groups stay resident: read again by the pair-1 V projection
        xv = [xp.tile([128, N_DT, NQ], fp16, tag=f"xv{g}", name=f"xv{g}")
              for g in range(4)]
        # single queue, exact consumption order (deterministic arbitration)
        nc.sync.dma_start(wk_sb[:], wk.rearrange("(t p) e -> p t e", p=128))
        nc.sync.dma_start(xk0[:], kTr[:, :, 0:NQ])
        nc.sync.dma_start(wq_sb[:], wq.rearrange("(t p) e -> p t e", p=128))
        nc.sync.dma_start(bk_sb[:], bk.rearrange("(m p) o -> p (m o)", p=128))
        nc.sync.dma_start(bq_sb[:], bq.rearrange("(m p) o -> p (m o)", p=128))
        nc.sync.dma_start(xq0[:], qTr[:, :, 0:NQ])
        nc.sync.dma_start(xk1[:], kTr[:, :, NQ:2 * NQ])
        nc.sync.dma_start(wv_sb[:], wv.rearrange("(t p) e -> p t e", p=128))
        nc.sync.dma_start(xv[0][:], vTr[:, :, 0:NQ])
        nc.sync.dma_start(xk2a, kTr[:, :, 2 * NQ:3 * NQ])
        nc.sync.dma_start(xv[1][:], vTr[:, :, NQ:2 * NQ])
        nc.sync.dma_start(xk2b, kTr[:, :, 3 * NQ:S])
        nc.sync.dma_start(xq1[:], qTr[:, :, NQ:2 * NQ])
        nc.sync.dma_start(xv[2][:], vTr[:, :, 2 * NQ:3 * NQ])
        nc.sync.dma_start(xv[3][:], vTr[:, :, 3 * NQ:S])
        nc.sync.dma_start(wo_sb[:], wo.rearrange("(t p) e -> p t e", p=128))
        nc.sync.dma_start(xq2a, qTr[:, :, 2 * NQ:3 * NQ])
        nc.sync.dma_start(xq2b, qTr[:, :, 3 * NQ:S])

        def k_rhs(n):
            if n == 0:
                return lambda d: xk0[:, d, :]
            if n == 1:
                return lambda d: xk1[:, d, :]
            return lambda d: xk2[:, d, (n - 2) * NQ:(n - 1) * NQ]

        def q_rhs(n):
            if n == 0:
                return lambda d: xq0[:, d, :]
            if n == 1:
                return lambda d: xq1[:, d, :]
            return lambda d: xq2[:, d, (n - 2) * NQ:(n - 1) * NQ]

        def proj_half(w_sb, b_sb, dst, m, n, rhs_of, half, cell):
            # half-round of a [e, s] projection (split for chore smoothing)
            if half == 0:
                cell.append(aux.tile([128, NQ], f32, tag="z", name="pz"))
            zb = cell[0]
            for d in (range(4) if half == 0 else range(4, N_DT)):
                nc.tensor.matmul(zb[:], w_sb[:, d, m * 128:(m + 1) * 128],
                                 rhs_of(d), start=(d == 0),
                                 stop=(d == N_DT - 1))
            if half == 1:
                nc.vector.tensor_scalar_add(
                    dst[m][:, n * NQ:(n + 1) * NQ], zb[:], b_sb[:, m:m + 1])

        def v_half(p, st, half, cell):
            # half of a V-projection s-tile -> vh[:, st, 2p:2p+2, :64]
            g, r = st // 4, st % 4
            if half == 0:
                cell.append(aux.tile([128, 2, DK], f32, tag="z", name="vb"))
            vb = cell[0]
            for d in (range(4) if half == 0 else range(4, N_DT)):
                nc.tensor.matmul(vb[:], xv[g][:, d, r * 128:(r + 1) * 128],
                                 wv_sb[:, d, p * 128:(p + 1) * 128],
                                 start=(d == 0), stop=(d == N_DT - 1))
            if half == 1:
                nc.vector.tensor_copy(vh[:, st, 2 * p:2 * p + 2, 0:DK], vb[:])

        def proj_round(w_sb, b_sb, dst, m, n, rhs_of):
            cell = []
            proj_half(w_sb, b_sb, dst, m, n, rhs_of, 0, cell)
            proj_half(w_sb, b_sb, dst, m, n, rhs_of, 1, cell)

        def kq_job(which, m, n, earliest):
            w_sb, b_sb, dst, rhs = ((wk_sb, bk_sb, kh, k_rhs(n))
                                    if which == "k" else
                                    (wq_sb, bq_sb, qh, q_rhs(n)))
            cell = []
            return {"earliest": earliest, "w": [0.89, 0.93], "atoms": [
                lambda h=0: proj_half(w_sb, b_sb, dst, m, n, rhs, 0, cell),
                lambda h=1: proj_half(w_sb, b_sb, dst, m, n, rhs, 1, cell)]}

        def v_job(p, st, earliest):
            cell = []
            return {"earliest": earliest, "w": [0.46, 0.5], "v_st": st,
                    "atoms": [
                        lambda: v_half(p, st, 0, cell),
                        lambda: v_half(p, st, 1, cell)]}

        def op_job(pqc, e2, earliest):
            cell = []
            return {"earliest": earliest, "w": [0.55, 0.55], "atoms": [
                lambda c=0: out_proj_c(pqc, otTs[pqc], e2, c, cell),
                lambda c=1: out_proj_c(pqc, otTs[pqc], e2, c, cell)]}

        def schedule(jobs):
            """Greedy per-slot packing of 2-atom jobs: at most two jobs'
            psum ring tiles open at once, atoms in order, per-slot budget."""
            BUDGET = 1.00
            chores, vh_rd = {}, {}
            open_q = []
            waiting = list(jobs)
            for slot in range(17):
                cap = 1e9 if slot == 16 else (BUDGET if slot != 1 else 1.0)
                while True:
                    si = next((i for i, j in enumerate(waiting)
                               if j["earliest"] <= slot or slot == 16), None)
                    if open_q and (len(open_q) >= 2 or si is None):
                        job, ai = open_q[0]
                        w = job["w"][ai]
                        if w > cap:
                            break
                        chores.setdefault(slot, []).append(job["atoms"][ai])
                        cap -= w
                        if ai + 1 == len(job["atoms"]):
                            open_q.pop(0)
                            if "v_st" in job:
                                vh_rd[job["v_st"]] = slot
                        else:
                            open_q[0] = (job, ai + 1)
                    elif si is not None:
                        job = waiting[si]
                        w = job["w"][0]
                        if w > cap:
                            break
                        waiting.pop(si)
                        chores.setdefault(slot, []).append(job["atoms"][0])
                        cap -= w
                        open_q.append((job, 1))
                    else:
                        break
            assert not open_q and not waiting, "chore overflow"
            return chores, vh_rd

        def out_proj_c(qc, otT, e2, c, cell):
            # one e-tile column (c) of out_proj for e-pair e2, q-chunk qc
            q0, q1 = qc * NQ, (qc + 1) * NQ
            if c == 0:
                cell.append(otp.tile([128, 2, NQ], fp16, tag="zs", name="zs",
                                     bufs=8))
            zs = cell[0]
            et = 2 * e2 + c
            zb = aux.tile([128, NQ], f32, tag="z", name="zb")
            nc.tensor.matmul(zb[:], wo_sb[:, 0, et * 128:(et + 1) * 128],
                             otT[:, 0, :], start=True, stop=False)
            nc.tensor.matmul(zb[:], wo_sb[:, 1, et * 128:(et + 1) * 128],
                             otT[:, 1, :], start=False, stop=True)
            nc.vector.tensor_copy(zs[:, c, :], zb[:])
            if c == 1:
                nc.sync.dma_start(zTv[:, 2 * e2:2 * e2 + 2, q0:q1], zs[:])

        def out_proj_tail(qc, otT, e2):
            # tail variant: zb pairs alternate between the free scores pool
            # and the aux pool (6 banks total), second copy on the idle ACT
            q0, q1 = qc * NQ, (qc + 1) * NQ
            zs = otp.tile([128, 2, NQ], fp16, tag="zs", name="zs", bufs=8)
            if e2 % 2 == 0:
                zb2 = scp.tile([128, 2, NQ], f32, tag="sc", name="zb2")
                zbs = [zb2[:, 0, :], zb2[:, 1, :]]
            else:
                zbs = [aux.tile([128, NQ], f32, tag="z", name="zba"),
                       aux.tile([128, NQ], f32, tag="z", name="zbb")]
            for c in range(2):
                et = 2 * e2 + c
                nc.tensor.matmul(zbs[c], wo_sb[:, 0, et * 128:(et + 1) * 128],
                                 otT[:, 0, :], start=True, stop=False)
                nc.tensor.matmul(zbs[c], wo_sb[:, 1, et * 128:(et + 1) * 128],
                                 otT[:, 1, :], start=False, stop=True)
                if c == 1:
                    nc.scalar.activation(zs[:, c, :], zbs[c], AF.Copy)
                else:
                    nc.vector.tensor_copy(zs[:, c, :], zbs[c])
            nc.sync.dma_start(zTv[:, 2 * e2:2 * e2 + 2, q0:q1], zs[:])

        # preload the exp table set while the first DMAs stream
        dummy = persist.tile([1, 1], fp16, tag="dummy", name="dummy")
        nc.scalar.activation(dummy[:], ones1[:1, 0:1], AF.Exp)

        def pe_warm(n):
            # keep the PE busy-run alive across DMA-gated gaps so real
            # matmuls are charged at the warm p-state
            for _ in range(n):
                tw = aux.tile([128, 128], fp16, tag="z", name="tw")
                nc.tensor.matmul(tw[:], ident[:], ident[:], is_transpose=True,
                                 start=True, stop=True, skip_group_check=True)

        # prologue: pair-0 K/Q chunk 0 (gates the first scores); K n1+ via
        # chores so a late k1 DMA cannot block Q n0 in the PE FIFO
        pe_warm(42)
        proj_round(wk_sb, bk_sb, kh, 0, 0, k_rhs(0))
        pe_warm(12)
        proj_round(wq_sb, bq_sb, qh, 0, 0, q_rhs(0))

        ots = [otp.tile([128, 4, E], fp16, tag="ot", name=f"ot{qc}")
               for qc in range(N_QC)]
        otTs = [otp.tile([128, 2, NQ], fp16, tag="otT", name=f"otT{qc}")
                for qc in range(N_QC)]
        vh_ready = [0] * N_KT  # slot (in current pass) after which vh is valid
        prev_norm = None       # deferred (normalize, transpose) of prior pass

        for pas in range(8):
            p, qc = pas // 4, pas % 4
            q0, q1 = qc * NQ, (qc + 1) * NQ
            ot, otT = ots[qc], otTs[qc]

            # job lists in priority order (earliest slots from DMA arrivals)
            jobs = []
            if pas == 0:
                jobs.append(kq_job("k", 0, 1, 1))
                jobs.append(kq_job("k", 0, 2, 5))
                jobs.append(kq_job("k", 0, 3, 7))
                jobs += [v_job(0, st, 4) for st in range(4)]
                jobs += [v_job(0, st, 6) for st in range(4, 8)]
                jobs += [v_job(0, st, 10) for st in range(8, 12)]
                jobs.append(kq_job("q", 0, 1, 9))
                jobs += [v_job(0, st, 11) for st in range(12, 16)]
            elif pas == 1:
                jobs.append(kq_job("q", 0, 2, 2))
                jobs.append(kq_job("k", 1, 0, 2))
                jobs += [v_job(1, st, 2) for st in range(8)]
            elif pas == 2:
                jobs.append(kq_job("q", 0, 3, 2))
                jobs.append(kq_job("k", 1, 1, 2))
                jobs += [v_job(1, st, 2) for st in range(8, 16)]
            elif pas == 3:
                jobs.append(kq_job("k", 1, 2, 2))
                jobs.append(kq_job("k", 1, 3, 2))
                jobs.append(kq_job("q", 1, 0, 2))
            elif pas == 4:
                jobs.append(kq_job("q", 1, 1, 2))
                jobs.append(kq_job("q", 1, 2, 8))
            else:
                pqc = qc - 1
                jobs += [op_job(pqc, e2, 2) for e2 in range(4)]
                if qc == 1:
                    jobs.append(kq_job("q", 1, 3, 8))
            chores, vh_rd = schedule(jobs)
            if pas == 0:
                for st in range(N_KT):
                    vh_ready[st] = vh_rd[st]
            else:
                vh_ready = [-1] * N_KT

            if prev_norm is not None:
                # previous pass's normalize+transpose runs in our first two
                # slots so its scores/exp never stall at the boundary
                chores.setdefault(0, []).insert(0, prev_norm[0])
                chores.setdefault(1, []).insert(0, prev_norm[1])
                prev_norm = None

            accs = [accp.tile([128, 4, DK + 1], f32, tag="acc",
                              name=f"acc{j}") for j in range(2)]
            pend = []   # kts whose P@V is not yet emitted

            def emit_pv(kt, pt):
                for j in range(2):
                    h = 2 * p + j
                    for qsb in range(4):
                        # start=True clears the whole bank row per written
                        # partition, so only the FIRST group in each acc bank
                        # may set it; later groups land on the cleared row
                        # (has_written=0 -> overwrite) and then accumulate.
                        nc.tensor.matmul(
                            accs[j][:, qsb, :],
                            pt[:, j * NQ + qsb * 128:j * NQ + (qsb + 1) * 128],
                            vh[:, kt, h, :], start=(kt == 0 and qsb == 0),
                            stop=(kt == N_KT - 1), skip_group_check=True)

            for kt in range(N_KT):
                sc = scp.tile([128, 2 * NQ], f32, tag="sc", name="sc")
                for j in range(2):
                    lo, hi = j * DK, (j + 1) * DK
                    nc.tensor.matmul(
                        sc[:, j * NQ:(j + 1) * NQ],
                        kh[p][lo:hi, kt * 128:(kt + 1) * 128],
                        qh[p][lo:hi, q0:q1], start=True, stop=True)
                pt = ptp.tile([128, 2 * NQ], fp16, tag="pt", name="pt")
                nc.scalar.activation(pt[:], sc[:], AF.Exp, scale=0.125)
                if _DEBUG and pas == 0 and kt in (0, 5, 10, 15):
                    dbg_pt = persist.tile([128, 2 * NQ], fp16,
                                          tag=f"dbgpt{kt}", name="dbgpt")
                    nc.vector.tensor_copy(dbg_pt[:], pt[:])
                    nc.sync.dma_start(dbg[f"pt{kt:02d}"][:, :], dbg_pt[:])
                for fn in chores.get(kt, ()):
                    fn()
                pend.append((kt, pt))
                # emit P@V for kts at least 1 slot old with vh available
                while pend and pend[0][0] < kt and vh_ready[pend[0][0]] <= kt:
                    emit_pv(*pend.pop(0))
            for fn in chores.get(16, ()):
                fn()
            for kt_, pt_ in pend:
                emit_pv(kt_, pt_)
            leftover = []

            if _DEBUG and pas == 0:
                dbg_acc = persist.tile([128, 4 * (DK + 1)], f32, tag="dbgacc",
                                       name="dbgacc")
                nc.vector.tensor_copy(
                    dbg_acc[:].rearrange("p (a b) -> p a b", a=4),
                    accs[0][:, :, :])
                nc.sync.dma_start(dbg["acc0"][:, :], dbg_acc[:])

            def norm_half(j, accs=accs, p=p, ot=ot, on_act=False):
                # ot[q, e] = acc[:, :, 0:64] * (1 / acc[:, :, 64])
                h = 2 * p + j
                rv = otp.tile([128, 4], f32, tag="rv", name="rv")
                nc.vector.reciprocal(rv[:], accs[j][:, :, DK])
                for qsb in range(4):
                    if on_act:
                        nc.scalar.activation(
                            ot[:, qsb, h * DK:(h + 1) * DK],
                            accs[j][:, qsb, 0:DK], AF.Copy,
                            scale=rv[:, qsb:qsb + 1])
                    else:
                        nc.vector.tensor_scalar_mul(
                            ot[:, qsb, h * DK:(h + 1) * DK],
                            accs[j][:, qsb, 0:DK], rv[:, qsb:qsb + 1])

            def transp(accs=accs, p=p, ot=ot, otT=otT):
                # this pair's half of ot -> otT on the PE (4 blocks, 1 bank)
                tp = aux.tile([128, 4 * 128], fp16, tag="z", name="tp")
                for qsb in range(4):
                    nc.tensor.matmul(tp[:, qsb * 128:(qsb + 1) * 128],
                                     ot[:, qsb, p * 128:(p + 1) * 128],
                                     ident[:], is_transpose=True, start=True,
                                     stop=True, skip_group_check=True)
                nc.vector.tensor_copy(otT[:, p, :], tp[:])

            prev_norm = (lambda nh=norm_half: (nh(0), nh(1)),
                         lambda t=transp: t(), norm_half,
                         lambda lo=leftover, ep=emit_pv:
                             [ep(k_, p_) for k_, p_ in lo])

        # last pass's normalize (split across DVE+ACT) + final out_proj
        prev_norm[2](0)
        prev_norm[2](1, on_act=True)
        prev_norm[1]()
        for e2 in range(4):
            out_proj_tail(N_QC - 1, otTs[N_QC - 1], e2)

        if _DEBUG:
            nc.sync.dma_start(dbg["kh0"][:, :], kh[0][:])
            nc.sync.dma_start(dbg["qh0"][:, :], qh[0][:])
            nc.sync.dma_start(
                dbg["vh"].rearrange("p (a b c) -> p a b c", b=HPC, c=DK + 1),
                vh[:, :, :, :])
            nc.sync.dma_start(
                dbg["ot0"].rearrange("p (a b) -> p a b", a=4), ots[0][:, :, :])
            nc.sync.dma_start(
                dbg["otT0"].rearrange("p (a b) -> p a b", a=2),
                otTs[0][:, :, :])

    nc.compile()
    return nc


def _get_program():
    global _PROGRAM
    if _PROGRAM is None:
        _PROGRAM = _build_program()
    return _PROGRAM


def _make_in_maps(q, k, v, Wq, bq, Wk, bk, Wv, Wo):
    f32 = np.float32
    f16 = np.float16
    xT = {}
    for b in range(B):
        xT[("q", b)] = np.ascontiguousarray(q[b].T, dtype=f16)
        xT[("k", b)] = np.ascontiguousarray(k[b].T, dtype=f16)
        xT[("v", b)] = np.ascontiguousarray(v[b].T, dtype=f16)
    ws = {}
    for g in range(4):
        sl = slice(g * E, (g + 1) * E)
        ws[("wq", g)] = np.ascontiguousarray(Wq[sl, :].T, dtype=f16)
        ws[("wk", g)] = np.ascontiguousarray(Wk[sl, :].T, dtype=f16)
        ws[("wv", g)] = np.ascontiguousarray(Wv[sl, :].T, dtype=f16)
        ws[("wo", g)] = np.ascontiguousarray(Wo[:, sl].T, dtype=f16)
        ws[("bq", g)] = np.ascontiguousarray(bq[sl].reshape(E, 1), dtype=f32)
        ws[("bk", g)] = np.ascontiguousarray(bk[sl].reshape(E, 1), dtype=f32)
    in_maps = []
    for c in range(N_CORES):
        b, g = c // 4, c % 4
        in_maps.append({
            "qT": xT[("q", b)], "kT": xT[("k", b)], "vT": xT[("v", b)],
            "wq": ws[("wq", g)], "wk": ws[("wk", g)], "wv": ws[("wv", g)],
            "wo": ws[("wo", g)], "bq": ws[("bq", g)], "bk": ws[("bk", g)],
        })
    return in_maps


def _numpy_fallback(q, k, v, mask, Wq, bq, Wk, bk, Wv, bv, Wo, bo):
    # Only used if mask is not all-True (never the case for this problem).
    def proj(x, W, b_):
        y = x @ W.T + b_
        return y.reshape(B, S, NUM_HEADS, DK).transpose(0, 2, 1, 3)
    qh, kh, vh = proj(q, Wq, bq), proj(k, Wk, bk), proj(v, Wv, bv)
    sc = np.einsum("bhqd,bhkd->bhqk", qh, kh) / np.sqrt(DK)
    sc = np.where(mask, sc, np.float32(-1e9))
    sc = sc - sc.max(-1, keepdims=True)
    p = np.exp(sc)
    p /= p.sum(-1, keepdims=True)
    o = np.einsum("bhqk,bhkd->bhqd", p, vh)
    o = o.transpose(0, 2, 1, 3).reshape(B, S, D_MODEL)
    return (o @ Wo.T + bo).astype(np.float32)


def kernel(q, k, v, mask, Wq, bq, Wk, bk, Wv, bv, Wo, bo):
    q = np.asarray(q, dtype=np.float32)
    k = np.asarray(k, dtype=np.float32)
    v = np.asarray(v, dtype=np.float32)
    Wq, Wk, Wv, Wo = (np.asarray(w, dtype=np.float32) for w in (Wq, Wk, Wv, Wo))
    bq, bk, bv, bo = (np.asarray(x, dtype=np.float32) for x in (bq, bk, bv, bo))
    if not np.all(np.asarray(mask)):
        return _numpy_fallback(q, k, v, np.asarray(mask), Wq, bq, Wk, bk,
                               Wv, bv, Wo, bo)

    from concourse.bass_utils import run_bass_kernel_spmd
    nc = _get_program()
    in_maps = _make_in_maps(q, k, v, Wq, bq, Wk, bk, Wv, Wo)
    res = run_bass_kernel_spmd(nc, in_maps, core_ids=list(range(N_CORES)),
                               **_RUN_KWARGS)
    global _LAST_RESULTS
    _LAST_RESULTS = res
    # V-bias folds out exactly: softmax rows sum to 1, so it contributes
    # bv @ Wo.T to every output row (added host-side with bo).
    bias_row = bo + bv @ Wo.T
    out = np.empty((B, S, D_MODEL), dtype=np.float32)
    for b in range(B):
        acc = res.results[4 * b]["zT"].astype(np.float32)
        for g in range(1, 4):
            acc = acc + res.results[4 * b + g]["zT"].astype(np.float32)
        out[b] = acc.T + bias_row
    return out
